# revision 1
# baseline (speedup 1.0000x reference)
"""Trainium2 Bass kernel for a pre-LN transformer block (MHA+RoPE, SiLU FFN).

Sharding: 8 cores; core c handles batch c//4, query block (c%4)*512..+512.
Each core redundantly computes LN1 + K/V for its whole batch (no collectives),
then attention/proj/FFN for its 512 queries. Inputs are column-rolled on the
host so every core's queries are token columns 0:512 of its xT input (SPMD
program identical across cores; RoPE tables rolled to match).

All activations live feature-major ([feature, token]); V is produced row-major
via an acts-stationary matmul so the attention AV contraction needs no
transposes. Softmax runs without max subtraction (scores are O(5) here), with
the denominator accumulated via a ones-column appended to V. RoPE's rotate-half
becomes an adjacent-pair swap (a 32-lane stream_shuffle) by permuting the q/k
weight columns on the host; q.k dot products are permutation-invariant.
LN gains/biases and the V bias are folded into adjacent weights on the host.
"""
import sys

sys.path.insert(0, "/opt/trn_rl_repo")

import numpy as np
import ml_dtypes

import concourse.bass as bass
import concourse.mybir as mybir
from concourse import bacc
from concourse.tile import TileContext
from concourse.bass_utils import run_bass_kernel_spmd

DIM, HEADS, B, T = 1024, 16, 2, 2048
HD = DIM // HEADS          # 64
NCORES = 8
CPB = NCORES // B          # cores per batch
QBLK = T // CPB            # 512 queries per core
ROPE_THETA = 10000.0
LN_EPS = 1e-5
KT = DIM // 128            # 8 feature tiles over DIM
NCH = T // 512             # 4 column chunks over T
RT = T // 128              # 16 key-row tiles

F32 = mybir.dt.float32
BF16 = mybir.dt.bfloat16
AF = mybir.ActivationFunctionType
OP = mybir.AluOpType

_bf = ml_dtypes.bfloat16


def _ln_finalize(nc, pool, ps_sum, ps_sq, eps_sb, tag):
    """From psum row-sums of x and x^2 over DIM, produce bf16 rstd and
    -mean*rstd rows ([1, 512])."""
    m_row = pool.tile([1, 512], F32, tag=f"{tag}m", name=f"{tag}_m")
    nc.scalar.mul(out=m_row[:], in_=ps_sum[:], mul=1.0 / DIM)
    msq = pool.tile([1, 512], F32, tag=f"{tag}msq", name=f"{tag}_msq")
    nc.scalar.mul(out=msq[:], in_=ps_sq[:], mul=1.0 / DIM)
    var = pool.tile([1, 512], F32, tag=f"{tag}var", name=f"{tag}_var")
    nc.vector.tensor_mul(out=var[:], in0=m_row[:], in1=m_row[:])
    nc.vector.tensor_sub(out=var[:], in0=msq[:], in1=var[:])
    std = pool.tile([1, 512], F32, tag=f"{tag}std", name=f"{tag}_std")
    nc.scalar.activation(out=std[:], in_=var[:], func=AF.Sqrt, bias=eps_sb[:])
    rs = pool.tile([1, 512], F32, tag=f"{tag}rs", name=f"{tag}_rs")
    nc.vector.reciprocal(out=rs[:], in_=std[:])
    nm = pool.tile([1, 512], F32, tag=f"{tag}nm", name=f"{tag}_nm")
    nc.vector.scalar_tensor_tensor(out=nm[:], in0=m_row[:], scalar=-1.0,
                                   in1=rs[:], op0=OP.mult, op1=OP.mult)
    rs_bf = pool.tile([1, 512], BF16, tag=f"{tag}rsbf", name=f"{tag}_rsbf")
    nc.scalar.copy(out=rs_bf[:], in_=rs[:])
    nm_bf = pool.tile([1, 512], BF16, tag=f"{tag}nmbf", name=f"{tag}_nmbf")
    nc.scalar.copy(out=nm_bf[:], in_=nm[:])
    return rs_bf, nm_bf


def _build_program():
    nc = bacc.Bacc("TRN2", target_bir_lowering=False, debug=False, num_devices=NCORES)

    xT = nc.declare_dram_parameter("xT", [DIM, T], F32, isOutput=False)
    cosd = nc.declare_dram_parameter("cosd", [128, T], BF16, isOutput=False)
    sind = nc.declare_dram_parameter("sind", [128, T], BF16, isOutput=False)
    Wq = nc.declare_dram_parameter("Wq", [DIM, DIM], BF16, isOutput=False)
    Wk = nc.declare_dram_parameter("Wk", [DIM, DIM], BF16, isOutput=False)
    Wv = nc.declare_dram_parameter("Wv", [DIM, DIM], BF16, isOutput=False)
    Wp = nc.declare_dram_parameter("Wp", [DIM, DIM], BF16, isOutput=False)
    W1 = nc.declare_dram_parameter("W1", [DIM, 4 * DIM], BF16, isOutput=False)
    W2 = nc.declare_dram_parameter("W2", [4 * DIM, DIM], BF16, isOutput=False)
    bq = nc.declare_dram_parameter("bq", [DIM], F32, isOutput=False)
    bk = nc.declare_dram_parameter("bk", [DIM], F32, isOutput=False)
    bp = nc.declare_dram_parameter("bp", [DIM], F32, isOutput=False)
    b1 = nc.declare_dram_parameter("b1", [4 * DIM], F32, isOutput=False)
    b2 = nc.declare_dram_parameter("b2", [DIM], F32, isOutput=False)
    outT = nc.declare_dram_parameter("outT", [DIM, QBLK], F32, isOutput=True)

    swap_mask = [j ^ 1 for j in range(32)]

    with TileContext(nc) as tc:
        with (
            tc.tile_pool(name="consts", bufs=1) as consts,
            tc.tile_pool(name="h1", bufs=KT) as h1p,
        ):
            ones_bf = consts.tile([128, 1], BF16)
            nc.vector.memset(ones_bf[:], 1.0)
            ones_row_bf = consts.tile([1, 128], BF16)
            nc.vector.memset(ones_row_bf[:], 1.0)
            ones_row_f = consts.tile([1, 128], F32)
            nc.vector.memset(ones_row_f[:], 1.0)
            eps_sb = consts.tile([1, 1], F32)
            nc.vector.memset(eps_sb[:], LN_EPS)
            cos_sb = consts.tile([128, T], BF16)
            sin_sb = consts.tile([128, T], BF16)
            nc.sync.dma_start(out=cos_sb[:], in_=cosd[:])
            nc.sync.dma_start(out=sin_sb[:], in_=sind[:])
            bq_sb = consts.tile([128, KT], F32)
            bk_sb = consts.tile([128, KT], F32)
            bp_sb = consts.tile([128, KT], F32)
            b1_sb = consts.tile([128, 4 * KT], F32)
            b2_sb = consts.tile([128, KT], F32)
            for dram, sb in ((bq, bq_sb), (bk, bk_sb), (bp, bp_sb),
                             (b1, b1_sb), (b2, b2_sb)):
                nc.sync.dma_start(out=sb[:], in_=dram.rearrange("(a p) -> p a", p=128))

            h1_tiles = []
            with (
                tc.tile_pool(name="ksb", bufs=KT) as ksbp,
                tc.tile_pool(name="vsb", bufs=RT) as vsbp,
                tc.tile_pool(name="qsb", bufs=KT) as qsbp,
            ):
                k_tiles, v_tiles, q_tiles = [], [], []
                with tc.tile_pool(name="nx", bufs=KT) as nxp:
                    nx_tiles = []
                    # ---- Phase 1: LN1 stats + apply (feature-major) ----
                    with (
                        tc.tile_pool(name="xk", bufs=3) as xkp,
                        tc.tile_pool(name="xbf", bufs=2) as xbfp,
                        tc.tile_pool(name="xsq", bufs=2) as xsqp,
                        tc.tile_pool(name="lnt", bufs=2) as lntp,
                        tc.tile_pool(name="bcast", bufs=1) as bcastp,
                        tc.tile_pool(name="stats", bufs=1) as statp,
                        tc.tile_pool(name="ps_st", bufs=NCH, space="PSUM") as ps_stp,
                    ):
                        ps_sums = [ps_stp.tile([1, 512], F32, tag="ps_sum",
                                               name=f"ps_sum_{n}") for n in range(NCH)]
                        ps_sqs = [ps_stp.tile([1, 512], F32, tag="ps_sq",
                                              name=f"ps_sq_{n}") for n in range(NCH)]
                        for k in range(KT):
                            xk = xkp.tile([128, T], F32, tag="xk")
                            nc.sync.dma_start(out=xk[:], in_=xT[k * 128:(k + 1) * 128, :])
                            xbf = xbfp.tile([128, T], BF16, tag="xbf")
                            nc.scalar.copy(out=xbf[:], in_=xk[:])
                            xsq = xsqp.tile([128, T], BF16, tag="xsq")
                            nc.scalar.square(out=xsq[:], in_=xk[:])
                            for n in range(NCH):
                                cs = slice(n * 512, (n + 1) * 512)
                                nc.tensor.matmul(ps_sums[n][:], ones_bf[:], xbf[:, cs],
                                                 start=(k == 0), stop=(k == KT - 1))
                                nc.tensor.matmul(ps_sqs[n][:], ones_bf[:], xsq[:, cs],
                                                 start=(k == 0), stop=(k == KT - 1))

                        rb_full = bcastp.tile([128, T], BF16)
                        mb_full = bcastp.tile([128, T], BF16)
                        for n in range(NCH):
                            cs = slice(n * 512, (n + 1) * 512)
                            rs_bf, nm_bf = _ln_finalize(nc, statp, ps_sums[n],
                                                        ps_sqs[n], eps_sb, "ln1")
                            psb = ps_stp.tile([128, 512], F32, tag="ps_sum",
                                              name="ps_bc_r")
                            nc.tensor.matmul(psb[:], ones_row_bf[:], rs_bf[:])
                            nc.scalar.copy(out=rb_full[:, cs], in_=psb[:])
                            psb2 = ps_stp.tile([128, 512], F32, tag="ps_sq",
                                               name="ps_bc_m")
                            nc.tensor.matmul(psb2[:], ones_row_bf[:], nm_bf[:])
                            nc.scalar.copy(out=mb_full[:, cs], in_=psb2[:])

                        for k in range(KT):
                            xk2 = xkp.tile([128, T], F32, tag="xk")
                            nc.sync.dma_start(out=xk2[:], in_=xT[k * 128:(k + 1) * 128, :])
                            t1 = lntp.tile([128, T], BF16, tag="lnt")
                            nc.vector.tensor_mul(out=t1[:], in0=xk2[:], in1=rb_full[:])
                            nx = nxp.tile([128, T], BF16, tag="nx")
                            nc.vector.tensor_add(out=nx[:], in0=t1[:], in1=mb_full[:])
                            nx_tiles.append(nx)

                    # ---- Phase 2: QKV ----
                    def rope_tile(ropep, dst, raw, cols):
                        n = cols.stop - cols.start
                        sh = ropep.tile([128, n], BF16, tag="rope_sh", name="rope_sh")
                        nc.vector.stream_shuffle(out=sh[:], in_=raw[:], mask=swap_mask)
                        t1 = ropep.tile([128, n], BF16, tag="rope_t1", name="rope_t1")
                        nc.vector.tensor_mul(out=t1[:], in0=raw[:], in1=cos_sb[:, cols])
                        t2 = ropep.tile([128, n], BF16, tag="rope_t2", name="rope_t2")
                        nc.vector.tensor_mul(out=t2[:], in0=sh[:], in1=sin_sb[:, cols])
                        nc.vector.tensor_add(out=dst, in0=t1[:], in1=t2[:])

                    with (
                        tc.tile_pool(name="wkq", bufs=2 * KT) as wkqp,
                        tc.tile_pool(name="rope", bufs=2) as ropep,
                        tc.tile_pool(name="ps_qk", bufs=4, space="PSUM") as ps_qkp,
                    ):
                        wk_t, wq_t = [], []
                        for k in range(KT):
                            w = wkqp.tile([128, DIM], BF16, tag="wk", name=f"wk_{k}")
                            nc.sync.dma_start(out=w[:], in_=Wk[k * 128:(k + 1) * 128, :])
                            wk_t.append(w)
                            w = wkqp.tile([128, DIM], BF16, tag="wq", name=f"wq_{k}")
                            nc.sync.dma_start(out=w[:], in_=Wq[k * 128:(k + 1) * 128, :])
                            wq_t.append(w)
                        for m in range(KT):
                            ms = slice(m * 128, (m + 1) * 128)
                            ksb = ksbp.tile([128, T], BF16, tag="ksb")
                            for n in range(NCH):
                                cs = slice(n * 512, (n + 1) * 512)
                                ps = ps_qkp.tile([128, 512], F32, tag="ps_k", name="ps_k")
                                for k in range(KT):
                                    nc.tensor.matmul(ps[:], wk_t[k][:, ms],
                                                     nx_tiles[k][:, cs],
                                                     start=(k == 0), stop=(k == KT - 1))
                                raw = ropep.tile([128, 512], BF16, tag="rope_raw",
                                                 name="rope_raw")
                                nc.scalar.activation(out=raw[:], in_=ps[:],
                                                     func=AF.Identity,
                                                     bias=bk_sb[:, m:m + 1])
                                rope_tile(ropep, ksb[:, cs], raw, cs)
                            k_tiles.append(ksb)

                            qsb = qsbp.tile([128, QBLK], BF16, tag="qsb")
                            ps = ps_qkp.tile([128, 512], F32, tag="ps_q", name="ps_q")
                            for k in range(KT):
                                nc.tensor.matmul(ps[:], wq_t[k][:, ms],
                                                 nx_tiles[k][:, 0:QBLK],
                                                 start=(k == 0), stop=(k == KT - 1))
                            raw = ropep.tile([128, 512], BF16, tag="rope_raw",
                                             name="rope_raw")
                            nc.scalar.activation(out=raw[:], in_=ps[:], func=AF.Identity,
                                                 bias=bq_sb[:, m:m + 1])
                            rope_tile(ropep, qsb[:], raw, slice(0, QBLK))
                            q_tiles.append(qsb)

                    # V row-major with interleaved ones columns
                    with (
                        tc.tile_pool(name="wv", bufs=KT) as wvp,
                        tc.tile_pool(name="ps_v", bufs=2, space="PSUM") as ps_vp,
                    ):
                        wv_t = []
                        for k in range(KT):
                            w = wvp.tile([128, DIM], BF16, tag="wv", name=f"wv_{k}")
                            nc.sync.dma_start(out=w[:], in_=Wv[k * 128:(k + 1) * 128, :])
                            wv_t.append(w)
                        for r in range(RT):
                            rs_ = slice(r * 128, (r + 1) * 128)
                            ps = ps_vp.tile([128, DIM], F32, tag="ps_v", name="ps_v")
                            for k in range(KT):
                                for vh in range(2):
                                    vs = slice(vh * 512, (vh + 1) * 512)
                                    nc.tensor.matmul(ps[:, vs], nx_tiles[k][:, rs_],
                                                     wv_t[k][:, vs],
                                                     start=(k == 0), stop=(k == KT - 1))
                            vsb = vsbp.tile([128, HEADS * (HD + 1)], BF16, tag="vsb")
                            v3 = vsb[:].rearrange("p (h c) -> p h c", c=HD + 1)
                            nc.any.tensor_copy(
                                v3[:, :, 0:HD],
                                ps[:].rearrange("p (h c) -> p h c", c=HD))
                            nc.vector.memset(v3[:, :, HD:HD + 1], 1.0)
                            v_tiles.append(vsb)

                # ---- Phase 3: attention ----
                with tc.tile_pool(name="avsb", bufs=KT) as avp:
                    av_tiles = []
                    with (
                        tc.tile_pool(name="esb", bufs=6) as esbp,
                        tc.tile_pool(name="asm", bufs=2) as asmp,
                        tc.tile_pool(name="ps_s", bufs=4, space="PSUM") as ps_sp,
                        tc.tile_pool(name="ps_av", bufs=2, space="PSUM") as ps_avp,
                        tc.tile_pool(name="ps_rb", bufs=2, space="PSUM") as ps_rbp,
                    ):
                        for f in range(HEADS // 2):
                            avsb = avp.tile([128, QBLK], BF16, tag="avsb")
                            ps_av = [ps_avp.tile([HD + 1, QBLK], F32, tag="ps_av",
                                                 name=f"ps_av_{f}_{i}")
                                     for i in range(2)]
                            for kt in range(RT):
                                kcs = slice(kt * 128, (kt + 1) * 128)
                                es = []
                                for half in range(2):
                                    hs = slice(half * HD, (half + 1) * HD)
                                    ps_s = ps_sp.tile([128, QBLK], F32, tag="ps_s",
                                                      name="ps_s")
                                    nc.tensor.matmul(ps_s[:], k_tiles[f][hs, kcs],
                                                     q_tiles[f][hs, :])
                                    e = esbp.tile([128, QBLK], BF16, tag="esb", name="esb")
                                    nc.scalar.activation(out=e[:], in_=ps_s[:],
                                                         func=AF.Exp,
                                                         scale=float(1.0 / np.sqrt(HD)))
                                    es.append(e)
                                for half in range(2):
                                    h = 2 * f + half
                                    nc.tensor.matmul(
                                        ps_av[half][:],
                                        v_tiles[kt][:, h * (HD + 1):(h + 1) * (HD + 1)],
                                        es[half][:],
                                        start=(kt == 0), stop=(kt == RT - 1))
                            for half in range(2):
                                r_row = asmp.tile([1, QBLK], F32, tag="r_row",
                                                  name="r_row")
                                nc.vector.reciprocal(out=r_row[:],
                                                     in_=ps_av[half][HD:HD + 1, :])
                                ps_rb = ps_rbp.tile([HD, QBLK], F32, tag="ps_rb",
                                                    name="ps_rb")
                                nc.tensor.matmul(ps_rb[:], ones_row_f[:, 0:HD],
                                                 r_row[:])
                                av_un = asmp.tile([HD, QBLK], BF16, tag="av_un",
                                                  name="av_un")
                                nc.scalar.copy(out=av_un[:], in_=ps_av[half][0:HD, :])
                                nc.vector.tensor_mul(
                                    out=avsb[half * HD:(half + 1) * HD, :],
                                    in0=av_un[:], in1=ps_rb[:])
                            av_tiles.append(avsb)

                    # ---- Phase 4: proj + bias + residual ----
                    with (
                        tc.tile_pool(name="wp", bufs=KT) as wpp,
                        tc.tile_pool(name="xq", bufs=2) as xqp,
                        tc.tile_pool(name="ps_p", bufs=3, space="PSUM") as ps_pp,
                    ):
                        wp_t = []
                        for k in range(KT):
                            w = wpp.tile([128, DIM], BF16, tag="wp", name=f"wp_{k}")
                            nc.sync.dma_start(out=w[:], in_=Wp[k * 128:(k + 1) * 128, :])
                            wp_t.append(w)
                        for m in range(KT):
                            ms = slice(m * 128, (m + 1) * 128)
                            xq = xqp.tile([128, QBLK], F32, tag="xq", name="xq")
                            nc.sync.dma_start(out=xq[:], in_=xT[ms, 0:QBLK])
                            ps = ps_pp.tile([128, QBLK], F32, tag="ps_p", name="ps_p")
                            for k in range(KT):
                                nc.tensor.matmul(ps[:], wp_t[k][:, ms], av_tiles[k][:],
                                                 start=(k == 0), stop=(k == KT - 1))
                            h1 = h1p.tile([128, QBLK], F32, tag="h1")
                            nc.vector.scalar_tensor_tensor(
                                out=h1[:], in0=ps[:], scalar=bp_sb[:, m:m + 1],
                                in1=xq[:], op0=OP.add, op1=OP.add)
                            h1_tiles.append(h1)

            # ---- Phase 5: LN2 ----
            with (
                tc.tile_pool(name="nx2", bufs=KT) as nx2p,
                tc.tile_pool(name="hbf", bufs=KT) as hbfp,
            ):
                nx2_tiles = []
                with (
                    tc.tile_pool(name="hsq", bufs=KT) as hsqp,
                    tc.tile_pool(name="ln2t", bufs=2) as ln2tp,
                    tc.tile_pool(name="stats2", bufs=1) as stat2p,
                    tc.tile_pool(name="bcast2", bufs=1) as bcast2p,
                    tc.tile_pool(name="ps_st2", bufs=2, space="PSUM") as ps_st2p,
                ):
                    hbf_tiles, hsq_tiles = [], []
                    for k in range(KT):
                        hbf = hbfp.tile([128, QBLK], BF16, tag="hbf")
                        nc.scalar.copy(out=hbf[:], in_=h1_tiles[k][:])
                        hsq = hsqp.tile([128, QBLK], BF16, tag="hsq")
                        nc.scalar.square(out=hsq[:], in_=h1_tiles[k][:])
                        hbf_tiles.append(hbf)
                        hsq_tiles.append(hsq)
                    ps_sum = ps_st2p.tile([1, 512], F32, tag="ps_sum2", name="ps_sum2")
                    ps_sq = ps_st2p.tile([1, 512], F32, tag="ps_sq2", name="ps_sq2")
                    for k in range(KT):
                        nc.tensor.matmul(ps_sum[:], ones_bf[:], hbf_tiles[k][:],
                                         start=(k == 0), stop=(k == KT - 1))
                        nc.tensor.matmul(ps_sq[:], ones_bf[:], hsq_tiles[k][:],
                                         start=(k == 0), stop=(k == KT - 1))
                    rs_bf, nm_bf = _ln_finalize(nc, stat2p, ps_sum, ps_sq, eps_sb, "ln2")
                    rb2 = bcast2p.tile([128, QBLK], BF16)
                    mb2 = bcast2p.tile([128, QBLK], BF16)
                    psb = ps_st2p.tile([128, 512], F32, tag="ps_sum2", name="ps_bc2r")
                    nc.tensor.matmul(psb[:], ones_row_bf[:], rs_bf[:])
                    nc.scalar.copy(out=rb2[:], in_=psb[:])
                    psb2 = ps_st2p.tile([128, 512], F32, tag="ps_sq2", name="ps_bc2m")
                    nc.tensor.matmul(psb2[:], ones_row_bf[:], nm_bf[:])
                    nc.scalar.copy(out=mb2[:], in_=psb2[:])
                    for k in range(KT):
                        t1 = ln2tp.tile([128, QBLK], BF16, tag="ln2t", name="ln2t")
                        nc.vector.tensor_mul(out=t1[:], in0=hbf_tiles[k][:], in1=rb2[:])
                        nx2 = nx2p.tile([128, QBLK], BF16, tag="nx2")
                        nc.vector.tensor_add(out=nx2[:], in0=t1[:], in1=mb2[:])
                        nx2_tiles.append(nx2)

                # ---- Phase 6: FFN1 + SiLU ----
                with tc.tile_pool(name="hs", bufs=4 * KT) as hsp:
                    hs_tiles = []
                    with (
                        tc.tile_pool(name="w1", bufs=KT) as w1p,
                        tc.tile_pool(name="ps_f", bufs=3, space="PSUM") as ps_fp,
                    ):
                        w1_t = []
                        for k in range(KT):
                            w = w1p.tile([128, 4 * DIM], BF16, tag="w1", name=f"w1_{k}")
                            nc.sync.dma_start(out=w[:], in_=W1[k * 128:(k + 1) * 128, :])
                            w1_t.append(w)
                        for m in range(4 * KT):
                            ms = slice(m * 128, (m + 1) * 128)
                            ps = ps_fp.tile([128, QBLK], F32, tag="ps_f", name="ps_f")
                            for k in range(KT):
                                nc.tensor.matmul(ps[:], w1_t[k][:, ms], nx2_tiles[k][:],
                                                 start=(k == 0), stop=(k == KT - 1))
                            hs = hsp.tile([128, QBLK], BF16, tag="hs", name="hs")
                            nc.scalar.activation(out=hs[:], in_=ps[:], func=AF.Silu,
                                                 bias=b1_sb[:, m:m + 1])
                            hs_tiles.append(hs)

                    # ---- Phase 7: FFN2 + bias + residual ----
                    with (
                        tc.tile_pool(name="w2", bufs=4) as w2p,
                        tc.tile_pool(name="osb", bufs=2) as osbp,
                        tc.tile_pool(name="ps_o", bufs=KT, space="PSUM") as ps_op,
                    ):
                        ps_o = [ps_op.tile([128, QBLK], F32, tag="ps_o", name=f"ps_o_{i}")
                                for i in range(KT)]
                        for k in range(4 * KT):
                            w2 = w2p.tile([128, DIM], BF16, tag="w2", name="w2")
                            nc.sync.dma_start(out=w2[:], in_=W2[k * 128:(k + 1) * 128, :])
                            for m in range(KT):
                                nc.tensor.matmul(ps_o[m][:], w2[:, m * 128:(m + 1) * 128],
                                                 hs_tiles[k][:],
                                                 start=(k == 0), stop=(k == 4 * KT - 1))
                        for m in range(KT):
                            osb = osbp.tile([128, QBLK], F32, tag="osb", name="osb")
                            nc.vector.scalar_tensor_tensor(
                                out=osb[:], in0=ps_o[m][:], scalar=b2_sb[:, m:m + 1],
                                in1=h1_tiles[m][:], op0=OP.add, op1=OP.add)
                            nc.sync.dma_start(out=outT[m * 128:(m + 1) * 128, :],
                                              in_=osb[:])

    nc.compile()
    return nc


_CACHE = {}


def _host_prep(inputs):
    g1 = np.asarray(inputs["ln1_g"], np.float32)
    b1v = np.asarray(inputs["ln1_b"], np.float32)
    g2 = np.asarray(inputs["ln2_g"], np.float32)
    b2v = np.asarray(inputs["ln2_b"], np.float32)
    W_qkv = np.asarray(inputs["W_qkv"], np.float32)
    b_qkv = np.asarray(inputs["b_qkv"], np.float32)
    W_proj = np.asarray(inputs["W_proj"], np.float32)
    b_proj = np.asarray(inputs["b_proj"], np.float32)
    W1 = np.asarray(inputs["W_ffn1"], np.float32)
    bf1 = np.asarray(inputs["b_ffn1"], np.float32)
    W2 = np.asarray(inputs["W_ffn2"], np.float32)
    bf2 = np.asarray(inputs["b_ffn2"], np.float32)

    Wf = g1[:, None] * W_qkv
    bf = b1v @ W_qkv + b_qkv
    Wq_, Wk_, Wv_ = Wf[:, :DIM], Wf[:, DIM:2 * DIM], Wf[:, 2 * DIM:]
    bq_, bk_, bv_ = bf[:DIM], bf[DIM:2 * DIM], bf[2 * DIM:]

    perm = np.empty(HD, np.int64)
    perm[0::2] = np.arange(HD // 2)
    perm[1::2] = np.arange(HD // 2) + HD // 2
    full_perm = np.concatenate([h * HD + perm for h in range(HEADS)])
    Wq_ = Wq_[:, full_perm]; bq_ = bq_[full_perm]
    Wk_ = Wk_[:, full_perm]; bk_ = bk_[full_perm]

    inv_freq = 1.0 / (ROPE_THETA ** (np.arange(0, HD, 2, dtype=np.float32) / HD))
    pos = np.arange(T, dtype=np.float32)
    ang = pos[None, :] * inv_freq[:, None]
    cosv = np.cos(ang).astype(np.float32)
    sinv = np.sin(ang).astype(np.float32)
    cos64 = np.repeat(cosv, 2, axis=0)
    sin64 = np.repeat(sinv, 2, axis=0).copy()
    sin64[0::2] *= -1.0
    cos2 = np.concatenate([cos64, cos64], axis=0).astype(_bf)
    sin2 = np.concatenate([sin64, sin64], axis=0).astype(_bf)

    bp_eff = b_proj + bv_ @ W_proj
    W1f = g2[:, None] * W1
    b1_eff = bf1 + b2v @ W1

    c = np.ascontiguousarray
    return dict(
        Wq=c(Wq_.astype(_bf)), Wk=c(Wk_.astype(_bf)), Wv=c(Wv_.astype(_bf)),
        Wp=c(W_proj.astype(_bf)), W1=c(W1f.astype(_bf)), W2=c(W2.astype(_bf)),
        bq=c(bq_), bk=c(bk_), bp=c(bp_eff), b1=c(b1_eff), b2=c(bf2),
        cos2=cos2, sin2=sin2,
    )


def make_in_maps(inputs):
    P = _host_prep(inputs)
    x = np.asarray(inputs["x"], np.float32)
    shared = {k: P[k] for k in ("Wq", "Wk", "Wv", "Wp", "W1", "W2",
                                "bq", "bk", "bp", "b1", "b2")}
    in_maps = []
    for c in range(NCORES):
        b = c // CPB
        qb = c % CPB
        roll = -qb * QBLK
        xTr = np.ascontiguousarray(np.roll(x[b].T, roll, axis=1))
        cosd = np.ascontiguousarray(np.roll(P["cos2"], roll, axis=1))
        sind = np.ascontiguousarray(np.roll(P["sin2"], roll, axis=1))
        in_maps.append(dict(shared, xT=xTr, cosd=cosd, sind=sind))
    return in_maps


def assemble_out(results):
    out = np.empty((B, T, DIM), np.float32)
    for c in range(NCORES):
        b = c // CPB
        qb = c % CPB
        out[b, qb * QBLK:(qb + 1) * QBLK, :] = results[c]["outT"].T
    return out


def get_program():
    if "nc" not in _CACHE:
        _CACHE["nc"] = _build_program()
    return _CACHE["nc"]


def kernel(**inputs):
    nc = get_program()
    in_maps = make_in_maps(inputs)
    res = run_bass_kernel_spmd(nc, in_maps, list(range(NCORES)))
    return assemble_out(res.results)



# revision 16
# speedup vs baseline: 1.2705x; 1.2705x over previous
"""Trainium2 Bass kernel for a pre-LN transformer block (MHA+RoPE, SiLU FFN).

Sharding: 8 cores; core c handles batch c//4, query block (c%4)*512..+512.
Each core redundantly computes LN1 + K/V for its whole batch (no collectives),
then attention/proj/FFN for its 512 queries. Inputs are column-rolled on the
host so every core's queries are token columns 0:512 of its xT input (SPMD
program identical across cores; RoPE tables rolled to match).

v2: fp8e4 DoubleRow matmuls for QKV/AV/proj (weights pre-scaled x64 on host,
unscaled in the PSUM-out step); q/k/es/v/av stored fp8; softmax without
max-subtraction but with a constant logit shift (C=4) so exp fits fp8 range;
x shipped bf16 (2-pass stream) with a separate f32 slice for the residual;
squares and half of RoPE on GpSimd; FFN stays bf16. RoPE's rotate-half is an
adjacent-pair swap (host-permuted q/k weight columns); LN gains/biases and the
V bias are folded into adjacent weights on the host.
"""
import sys

sys.path.insert(0, "/opt/trn_rl_repo")

import numpy as np
import ml_dtypes

import concourse.bass as bass
import concourse.mybir as mybir
from concourse import bacc
from concourse.tile import TileContext
from concourse.bass_utils import run_bass_kernel_spmd

DIM, HEADS, B, T = 1024, 16, 2, 2048
HD = DIM // HEADS          # 64
NCORES = 8
CPB = NCORES // B          # cores per batch
QBLK = T // CPB            # 512 queries per core
ROPE_THETA = 10000.0
LN_EPS = 1e-5
KT = DIM // 128            # 8 feature tiles over DIM
NCH = T // 512             # 4 column chunks over T
RT = T // 128              # 16 key-row tiles
NP = KT // 2               # 4 DoubleRow k-pairs over DIM

WS = 64.0                  # fp8 weight pre-scale
CSHIFT = 4.0               # softmax logit shift (max logit ~7.3)

F32 = mybir.dt.float32
BF16 = mybir.dt.bfloat16
F8 = mybir.dt.float8e4
AF = mybir.ActivationFunctionType
OP = mybir.AluOpType
DR = mybir.MatmulPerfMode.DoubleRow

_bf = ml_dtypes.bfloat16
_f8 = ml_dtypes.float8_e4m3


def _ln_finalize(nc, pool, ps_sum, ps_sq, eps_sb, tag):
    """From psum row-sums of x and x^2 over DIM, produce bf16 rstd and
    -mean*rstd rows ([1, 512])."""
    m_row = pool.tile([1, 512], F32, tag="lnf_m", name=f"{tag}_m")
    nc.scalar.mul(out=m_row[:], in_=ps_sum[:], mul=1.0 / DIM)
    msq = pool.tile([1, 512], F32, tag="lnf_msq", name=f"{tag}_msq")
    nc.scalar.mul(out=msq[:], in_=ps_sq[:], mul=1.0 / DIM)
    var = pool.tile([1, 512], F32, tag="lnf_var", name=f"{tag}_var")
    nc.vector.tensor_mul(out=var[:], in0=m_row[:], in1=m_row[:])
    nc.vector.tensor_sub(out=var[:], in0=msq[:], in1=var[:])
    std = pool.tile([1, 512], F32, tag="lnf_std", name=f"{tag}_std")
    nc.scalar.activation(out=std[:], in_=var[:], func=AF.Sqrt, bias=eps_sb[:])
    rs = pool.tile([1, 512], F32, tag="lnf_rs", name=f"{tag}_rs")
    nc.vector.reciprocal_approx_fast(out=rs[:], in_=std[:])
    nm = pool.tile([1, 512], F32, tag="lnf_nm", name=f"{tag}_nm")
    nc.vector.scalar_tensor_tensor(out=nm[:], in0=m_row[:], scalar=-1.0,
                                   in1=rs[:], op0=OP.mult, op1=OP.mult)
    rs_bf = pool.tile([1, 512], BF16, tag="lnf_rsbf", name=f"{tag}_rsbf")
    nc.vector.tensor_copy(rs_bf[:], rs[:])
    nm_bf = pool.tile([1, 512], BF16, tag="lnf_nmbf", name=f"{tag}_nmbf")
    nc.vector.tensor_copy(nm_bf[:], nm[:])
    return rs_bf, nm_bf


def _phase1_ln1(nc, tc, ctx):
    """Two-pass stream of bf16 x: stats (PE ones-matmuls), batched finalize,
    broadcast, then re-stream + apply -> nx_all fp8."""
    xT, nx_all = ctx["xT"], ctx["nx_all"]
    ones_bf, ones_row_bf, eps1 = ctx["ones_bf"], ctx["ones_row_bf"], ctx["eps1"]
    with (
        tc.tile_pool(name="xbf", bufs=2) as xbfp,
        tc.tile_pool(name="xsq", bufs=2) as xsqp,
        tc.tile_pool(name="lnt", bufs=2) as lntp,
        tc.tile_pool(name="stats", bufs=1) as statp,
        tc.tile_pool(name="bcast", bufs=1) as bcastp,
        tc.tile_pool(name="ps_st", bufs=NCH, space="PSUM") as ps_stp,
    ):
        ps_sums = [ps_stp.tile([1, 512], F32, tag="ps_sum",
                               name=f"ps_sum_{n}") for n in range(NCH)]
        ps_sqs = [ps_stp.tile([1, 512], F32, tag="ps_sq",
                              name=f"ps_sq_{n}") for n in range(NCH)]
        for k in range(KT):
            xbf = xbfp.tile([128, T], BF16, tag="xbf")
            nc.sync.dma_start(out=xbf[:], in_=xT[k * 128:(k + 1) * 128, :])
            for n in range(NCH):
                cs = slice(n * 512, (n + 1) * 512)
                xsq = xsqp.tile([128, 512], BF16, tag="xsq")
                nc.gpsimd.tensor_mul(out=xsq[:], in0=xbf[:, cs], in1=xbf[:, cs])
                nc.tensor.matmul(ps_sums[n][:], ones_bf[:], xbf[:, cs],
                                 start=(k == 0), stop=(k == KT - 1))
                nc.tensor.matmul(ps_sqs[n][:], ones_bf[:], xsq[:],
                                 start=(k == 0), stop=(k == KT - 1))

        rb_full = bcastp.tile([128, T], BF16)
        mb_full = bcastp.tile([128, T], BF16)
        for n in range(NCH):
            cs = slice(n * 512, (n + 1) * 512)
            rs_bf, nm_bf = _ln_finalize(nc, statp, ps_sums[n], ps_sqs[n],
                                        eps1, f"ln1_{n}")
            psb = ps_stp.tile([128, 512], F32, tag="ps_sum", name="ps_bc_r")
            nc.tensor.matmul(psb[:], ones_row_bf[:], rs_bf[:])
            nc.scalar.copy(out=rb_full[:, cs], in_=psb[:])
            psb2 = ps_stp.tile([128, 512], F32, tag="ps_sq", name="ps_bc_m")
            nc.tensor.matmul(psb2[:], ones_row_bf[:], nm_bf[:])
            nc.scalar.copy(out=mb_full[:, cs], in_=psb2[:])

        # pass 2: re-stream x, apply LN -> nx fp8
        for k in range(KT):
            xbf2 = xbfp.tile([128, T], BF16, tag="xbf")
            nc.sync.dma_start(out=xbf2[:], in_=xT[k * 128:(k + 1) * 128, :])
            for n in range(NCH):
                cs = slice(n * 512, (n + 1) * 512)
                t1 = lntp.tile([128, 512], BF16, tag="lnt")
                nc.vector.tensor_mul(out=t1[:], in0=xbf2[:, cs], in1=rb_full[:, cs])
                nc.vector.tensor_add(out=nx_all[:, k * T + n * 512:
                                                k * T + (n + 1) * 512],
                                     in0=t1[:], in1=mb_full[:, cs])


def _phase2_qkv(nc, tc, ctx):
    """Q/K/V via fp8 DoubleRow; RoPE on DVE+GpSimd; V fp8 + ones column.
    Also loads W1, and builds xqb (f32 residual + proj bias)."""
    Wq8, Wk8, Wv8, W1, xq = ctx["Wq8"], ctx["Wk8"], ctx["Wv8"], ctx["W1"], ctx["xq"]
    nx_all, v_all = ctx["nx_all"], ctx["v_all"]
    cos_sb, sin_sb = ctx["cos_sb"], ctx["sin_sb"]
    bq_sb, bk_sb, bp_sb = ctx["bq_sb"], ctx["bk_sb"], ctx["bp_sb"]
    k_tiles, q_tiles = ctx["k_tiles"], ctx["q_tiles"]
    w1_t, xqb_tiles = ctx["w1_t"], ctx["xqb_tiles"]
    ksbp, qsbp, w1p, xqbp = ctx["ksbp"], ctx["qsbp"], ctx["w1p"], ctx["xqbp"]
    swap_mask = [j ^ 1 for j in range(32)]
    nxv = nx_all[:].rearrange("p (j t) -> p j t", t=T)

    with (
        tc.tile_pool(name="wqk8", bufs=NP) as wqk8p,
        tc.tile_pool(name="wv8", bufs=NP) as wv8p,
        tc.tile_pool(name="rope", bufs=2) as ropep,
        tc.tile_pool(name="ps_qk", bufs=3, space="PSUM") as ps_qkp,
        tc.tile_pool(name="ps_v", bufs=2, space="PSUM") as ps_vp,
    ):
        wk8_t, wq8_t, wv8_t = [], [], []
        for t in range(NP):
            w = wqk8p.tile([128, 2 * DIM], F8, tag="wk8", name=f"wk8_{t}")
            nc.sync.dma_start(out=w[:], in_=Wk8[t * 128:(t + 1) * 128, :])
            wk8_t.append(w)
            w = wqk8p.tile([128, 2 * DIM], F8, tag="wq8", name=f"wq8_{t}")
            nc.sync.dma_start(out=w[:], in_=Wq8[t * 128:(t + 1) * 128, :])
            wq8_t.append(w)
        for t in range(NP):
            w = wv8p.tile([128, 2 * DIM], F8, tag="wv8", name=f"wv8_{t}")
            nc.sync.dma_start(out=w[:], in_=Wv8[t * 128:(t + 1) * 128, :])
            wv8_t.append(w)

        def rope_tile(dst, raw, cols):
            n = cols.stop - cols.start
            sh = ropep.tile([128, n], BF16, tag="rope_sh", name="rope_sh")
            nc.vector.stream_shuffle(out=sh[:], in_=raw[:], mask=swap_mask)
            t1 = ropep.tile([128, n], BF16, tag="rope_t1", name="rope_t1")
            nc.gpsimd.tensor_mul(out=t1[:], in0=raw[:], in1=cos_sb[:, cols])
            t2 = ropep.tile([128, n], BF16, tag="rope_t2", name="rope_t2")
            nc.gpsimd.tensor_mul(out=t2[:], in0=sh[:], in1=sin_sb[:, cols])
            nc.vector.tensor_add(out=dst, in0=t1[:], in1=t2[:])

        def qk_tile(w8_t, b_sb, m, dst, cs):
            ps = ps_qkp.tile([128, 512], F32, tag="ps_qk", name="ps_qk")
            ms = slice(m * 128, (m + 1) * 128)
            for t in range(NP):
                w_ap = w8_t[t][:].rearrange("p (j m) -> p j m", j=2)[:, :, ms]
                nc.tensor.matmul(ps[:], w_ap, nxv[:, 2 * t:2 * t + 2, cs],
                                 start=(t == 0), stop=(t == NP - 1), perf_mode=DR)
            raw = ropep.tile([128, 512], BF16, tag="rope_raw", name="rope_raw")
            nc.scalar.activation(out=raw[:], in_=ps[:], func=AF.Identity,
                                 scale=1.0 / WS, bias=b_sb[:, m:m + 1])
            rope_tile(dst, raw, cs)

        for m in range(KT):
            ksb = ksbp.tile([128, T], F8, tag="ksb")
            for n in range(NCH):
                cs = slice(n * 512, (n + 1) * 512)
                qk_tile(wk8_t, bk_sb, m, ksb[:, cs], cs)
            k_tiles.append(ksb)
            qsb = qsbp.tile([128, QBLK], F8, tag="qsb")
            qk_tile(wq8_t, bq_sb, m, qsb[:], slice(0, QBLK))
            q_tiles.append(qsb)

        # W1 prefetch + xq residual load (after QKV-critical DMAs)
        for k in range(KT):
            w = w1p.tile([128, 4 * DIM], BF16, tag="w1", name=f"w1_{k}")
            nc.sync.dma_start(out=w[:], in_=W1[k * 128:(k + 1) * 128, :])
            w1_t.append(w)
            xqb = xqbp.tile([128, QBLK], F32, tag="xqb", name=f"xqb_{k}")
            nc.sync.dma_start(out=xqb[:], in_=xq[k * 128:(k + 1) * 128, :])
            nc.vector.tensor_scalar_add(xqb[:], xqb[:], bp_sb[:, k:k + 1])
            xqb_tiles.append(xqb)

        # V (fp8 DoubleRow, acts-stationary), fp8 out + ones column
        vv4 = v_all[:].rearrange("p (r h c) -> p r h c", h=HEADS, c=HD + 1)
        for r in range(RT):
            rs_ = slice(r * 128, (r + 1) * 128)
            ps = ps_vp.tile([128, DIM], F32, tag="ps_v", name="ps_v")
            for vh in range(2):
                vs = slice(vh * 512, (vh + 1) * 512)
                for t in range(NP):
                    w_ap = wv8_t[t][:].rearrange("p (j m) -> p j m", j=2)[:, :, vs]
                    nc.tensor.matmul(ps[:, vs], nxv[:, 2 * t:2 * t + 2, rs_],
                                     w_ap, start=(t == 0), stop=(t == NP - 1),
                                     perf_mode=DR)
            nc.scalar.mul(out=vv4[:, r, :, 0:HD],
                          in_=ps[:].rearrange("p (h c) -> p h c", c=HD),
                          mul=1.0 / WS)
            nc.vector.memset(vv4[:, r, :, HD:HD + 1], 1.0)


def _phase3_attention(nc, tc, ctx):
    """Scores (fp8 x fp8, paired PSUM) -> exp (fp8, shifted) -> AV DoubleRow
    with ones-column denominators -> normalize -> av_all fp8."""
    v_all, av_all = ctx["v_all"], ctx["av_all"]
    k_tiles, q_tiles = ctx["k_tiles"], ctx["q_tiles"]
    ones_row_f, negc = ctx["ones_row_f"], ctx["negc"]

    with (
        tc.tile_pool(name="es", bufs=2) as esp,
        tc.tile_pool(name="rrow", bufs=2) as rrowp,
        tc.tile_pool(name="avun", bufs=2) as avunp,
        tc.tile_pool(name="ps_s", bufs=2, space="PSUM") as ps_sp,
        tc.tile_pool(name="ps_av", bufs=2, space="PSUM") as ps_avp,
        tc.tile_pool(name="ps_rb", bufs=2, space="PSUM") as ps_rbp,
    ):
        vv3 = v_all[:].rearrange("p (r hc) -> p r hc", hc=HEADS * (HD + 1))

        def scores_half(f, half, es_t):
            hsl = slice(half * HD, (half + 1) * HD)
            for rp in range(RT // 2):
                ps_s = ps_sp.tile([128, 1024], F32, tag="ps_s", name="ps_s")
                for j in range(2):
                    kt = 2 * rp + j
                    kcs = slice(kt * 128, (kt + 1) * 128)
                    nc.tensor.matmul(ps_s[:, j * 512:(j + 1) * 512],
                                     k_tiles[f][hsl, kcs], q_tiles[f][hsl, :])
                nc.scalar.activation(
                    out=es_t[half][:, rp * 1024:(rp + 1) * 1024],
                    in_=ps_s[:], func=AF.Exp,
                    scale=float(1.0 / np.sqrt(HD)), bias=negc[:])

        def av_half(f, half, es_t):
            h = 2 * f + half
            esv = es_t[half][:].rearrange("p (r n) -> p r n", n=QBLK)
            ps_av = ps_avp.tile([HD + 1, QBLK], F32, tag="ps_av",
                                name=f"ps_av_{f}_{half}")
            for rp in range(RT // 2):
                nc.tensor.matmul(
                    ps_av[:],
                    vv3[:, 2 * rp:2 * rp + 2, h * (HD + 1):(h + 1) * (HD + 1)],
                    esv[:, 2 * rp:2 * rp + 2, :],
                    start=(rp == 0), stop=(rp == RT // 2 - 1), perf_mode=DR)
            den = rrowp.tile([1, QBLK], F32, tag="den", name="den")
            nc.vector.tensor_copy(den[:], ps_av[HD:HD + 1, :])
            r_row = rrowp.tile([1, QBLK], F32, tag="r_row", name="r_row")
            nc.vector.reciprocal_approx_fast(out=r_row[:], in_=den[:])
            ps_rb = ps_rbp.tile([HD, QBLK], F32, tag="ps_rb", name="ps_rb")
            nc.tensor.matmul(ps_rb[:], ones_row_f[:], r_row[:])
            av_un = avunp.tile([HD, QBLK], BF16, tag="av_un", name="av_un")
            nc.vector.tensor_copy(av_un[:], ps_av[0:HD, :])
            nc.vector.tensor_mul(
                out=av_all[half * HD:(half + 1) * HD, f * QBLK:(f + 1) * QBLK],
                in0=av_un[:], in1=ps_rb[:])

        for f in range(HEADS // 2):
            es_t = [esp.tile([128, RT * QBLK], F8, tag=f"es{half}",
                             name=f"es_{f}_{half}") for half in range(2)]
            for half in range(2):
                scores_half(f, half, es_t)
            for half in range(2):
                av_half(f, half, es_t)


def _phase4_proj(nc, tc, ctx):
    """proj (fp8 DoubleRow) + bias + residual -> h1 f32."""
    av_all, wp8_t = ctx["av_all"], ctx["wp8_t"]
    xqb_tiles, h1_tiles, h1p = ctx["xqb_tiles"], ctx["h1_tiles"], ctx["h1p"]
    avv = av_all[:].rearrange("p (j n) -> p j n", n=QBLK)
    with tc.tile_pool(name="ps_p", bufs=3, space="PSUM") as ps_pp:
        for m in range(KT):
            ms = slice(m * 128, (m + 1) * 128)
            ps = ps_pp.tile([128, QBLK], F32, tag="ps_p", name="ps_p")
            for t in range(NP):
                w_ap = wp8_t[t][:].rearrange("p (j m) -> p j m", j=2)[:, :, ms]
                nc.tensor.matmul(ps[:], w_ap, avv[:, 2 * t:2 * t + 2, :],
                                 start=(t == 0), stop=(t == NP - 1), perf_mode=DR)
            h1 = h1p.tile([128, QBLK], F32, tag="h1", name=f"h1_{m}")
            nc.vector.scalar_tensor_tensor(
                out=h1[:], in0=ps[:], scalar=1.0 / WS,
                in1=xqb_tiles[m][:], op0=OP.mult, op1=OP.add)
            h1_tiles.append(h1)


def _phase5_ln2(nc, tc, ctx, nx2_tiles):
    """LN2: bf16 copies + gpsimd squares, PE stat sums, apply -> nx2 bf16."""
    h1_tiles = ctx["h1_tiles"]
    ones_bf, ones_row_bf, eps1 = ctx["ones_bf"], ctx["ones_row_bf"], ctx["eps1"]
    nx2p = ctx["nx2p"]
    with (
        tc.tile_pool(name="hbf", bufs=2) as hbfp,
        tc.tile_pool(name="hsq", bufs=2) as hsqp,
        tc.tile_pool(name="ln2t", bufs=2) as ln2tp,
        tc.tile_pool(name="stats2", bufs=1) as stat2p,
        tc.tile_pool(name="bcast2", bufs=1) as bcast2p,
        tc.tile_pool(name="ps_st2", bufs=2, space="PSUM") as ps_st2p,
    ):
        ps_sum = ps_st2p.tile([1, 512], F32, tag="ps_sum2", name="ps_sum2")
        ps_sq = ps_st2p.tile([1, 512], F32, tag="ps_sq2", name="ps_sq2")
        for k in range(KT):
            hbf = hbfp.tile([128, QBLK], BF16, tag="hbf")
            nc.vector.tensor_copy(hbf[:], h1_tiles[k][:])
            hsq = hsqp.tile([128, QBLK], BF16, tag="hsq")
            nc.gpsimd.tensor_mul(out=hsq[:], in0=hbf[:], in1=hbf[:])
            nc.tensor.matmul(ps_sum[:], ones_bf[:], hbf[:],
                             start=(k == 0), stop=(k == KT - 1))
            nc.tensor.matmul(ps_sq[:], ones_bf[:], hsq[:],
                             start=(k == 0), stop=(k == KT - 1))
        rs_bf2, nm_bf2 = _ln_finalize(nc, stat2p, ps_sum, ps_sq, eps1, "ln2")
        rb2 = bcast2p.tile([128, QBLK], BF16)
        mb2 = bcast2p.tile([128, QBLK], BF16)
        psb = ps_st2p.tile([128, 512], F32, tag="ps_sum2", name="ps_bc2r")
        nc.tensor.matmul(psb[:], ones_row_bf[:], rs_bf2[:])
        nc.scalar.copy(out=rb2[:], in_=psb[:])
        psb2 = ps_st2p.tile([128, 512], F32, tag="ps_sq2", name="ps_bc2m")
        nc.tensor.matmul(psb2[:], ones_row_bf[:], nm_bf2[:])
        nc.scalar.copy(out=mb2[:], in_=psb2[:])
        for k in range(KT):
            t1 = ln2tp.tile([128, QBLK], BF16, tag="ln2t", name="ln2t")
            nc.vector.tensor_mul(out=t1[:], in0=h1_tiles[k][:], in1=rb2[:])
            nx2 = nx2p.tile([128, QBLK], BF16, tag="nx2")
            nc.vector.tensor_add(out=nx2[:], in0=t1[:], in1=mb2[:])
            nx2_tiles.append(nx2)


def _phase6_ffn(nc, tc, ctx, nx2_tiles):
    """FFN1 (bf16, resident W1) + SiLU; FFN2 (bf16, streamed W2) + residual."""
    W2, outT = ctx["W2"], ctx["outT"]
    w1_t, h1_tiles = ctx["w1_t"], ctx["h1_tiles"]
    b1_sb, b2_sb, hsp = ctx["b1_sb"], ctx["b2_sb"], ctx["hsp"]
    hs_tiles = []
    with tc.tile_pool(name="ps_f", bufs=3, space="PSUM") as ps_fp:
        for m in range(4 * KT):
            ms = slice(m * 128, (m + 1) * 128)
            ps = ps_fp.tile([128, QBLK], F32, tag="ps_f", name="ps_f")
            for k in range(KT):
                nc.tensor.matmul(ps[:], w1_t[k][:, ms], nx2_tiles[k][:],
                                 start=(k == 0), stop=(k == KT - 1))
            hs = hsp.tile([128, QBLK], BF16, tag="hs", name="hs")
            nc.scalar.activation(out=hs[:], in_=ps[:], func=AF.Silu,
                                 bias=b1_sb[:, m:m + 1])
            hs_tiles.append(hs)

    with (
        tc.tile_pool(name="w2", bufs=10) as w2p,
        tc.tile_pool(name="osb", bufs=2) as osbp,
        tc.tile_pool(name="ps_o", bufs=KT, space="PSUM") as ps_op,
    ):
        ps_o = [ps_op.tile([128, QBLK], F32, tag="ps_o", name=f"ps_o_{i}")
                for i in range(KT)]
        for k in range(4 * KT):
            w2 = w2p.tile([128, DIM], BF16, tag="w2", name="w2")
            nc.sync.dma_start(out=w2[:], in_=W2[k * 128:(k + 1) * 128, :])
            for m in range(KT):
                nc.tensor.matmul(ps_o[m][:], w2[:, m * 128:(m + 1) * 128],
                                 hs_tiles[k][:],
                                 start=(k == 0), stop=(k == 4 * KT - 1))
        for m in range(KT):
            osb = osbp.tile([128, QBLK], F32, tag="osb", name="osb")
            nc.vector.scalar_tensor_tensor(
                out=osb[:], in0=ps_o[m][:], scalar=b2_sb[:, m:m + 1],
                in1=h1_tiles[m][:], op0=OP.add, op1=OP.add)
            nc.sync.dma_start(out=outT[m * 128:(m + 1) * 128, :], in_=osb[:])


def _build_program():
    nc = bacc.Bacc("TRN2", target_bir_lowering=False, debug=False,
                   num_devices=NCORES)

    ctx = {}
    ctx["xT"] = nc.declare_dram_parameter("xT", [DIM, T], BF16, isOutput=False)
    ctx["xq"] = nc.declare_dram_parameter("xq", [DIM, QBLK], F32, isOutput=False)
    cosd = nc.declare_dram_parameter("cosd", [128, T], BF16, isOutput=False)
    sind = nc.declare_dram_parameter("sind", [128, T], BF16, isOutput=False)
    for nm in ("Wq8", "Wk8", "Wv8", "Wp8"):
        ctx[nm] = nc.declare_dram_parameter(nm, [NP * 128, 2 * DIM], F8,
                                            isOutput=False)
    ctx["W1"] = nc.declare_dram_parameter("W1", [DIM, 4 * DIM], BF16,
                                          isOutput=False)
    ctx["W2"] = nc.declare_dram_parameter("W2", [4 * DIM, DIM], BF16,
                                          isOutput=False)
    bq = nc.declare_dram_parameter("bq", [DIM], F32, isOutput=False)
    bk = nc.declare_dram_parameter("bk", [DIM], F32, isOutput=False)
    bp = nc.declare_dram_parameter("bp", [DIM], F32, isOutput=False)
    b1 = nc.declare_dram_parameter("b1", [4 * DIM], F32, isOutput=False)
    b2 = nc.declare_dram_parameter("b2", [DIM], F32, isOutput=False)
    ctx["outT"] = nc.declare_dram_parameter("outT", [DIM, QBLK], F32,
                                            isOutput=True)

    with TileContext(nc) as tc:
        with (
            tc.tile_pool(name="consts", bufs=1) as consts,
            tc.tile_pool(name="xqb", bufs=KT) as xqbp,
            tc.tile_pool(name="h1", bufs=KT) as h1p,
            tc.tile_pool(name="avall", bufs=1) as avallp,
            tc.tile_pool(name="wp8", bufs=NP) as wp8p,
            tc.tile_pool(name="w1", bufs=KT) as w1p,
        ):
            ones_bf = consts.tile([128, 1], BF16)
            nc.vector.memset(ones_bf[:], 1.0)
            ones_row_bf = consts.tile([1, 128], BF16)
            nc.vector.memset(ones_row_bf[:], 1.0)
            ones_row_f = consts.tile([1, HD], F32)
            nc.vector.memset(ones_row_f[:], 1.0)
            eps4 = consts.tile([NCH, 1], F32)
            nc.vector.memset(eps4[:], LN_EPS)
            eps1 = consts.tile([1, 1], F32)
            nc.vector.memset(eps1[:], LN_EPS)
            negc = consts.tile([128, 1], F32)
            nc.vector.memset(negc[:], -CSHIFT)
            bq_sb = consts.tile([128, KT], F32)
            bk_sb = consts.tile([128, KT], F32)
            bp_sb = consts.tile([128, KT], F32)
            b1_sb = consts.tile([128, 4 * KT], F32)
            b2_sb = consts.tile([128, KT], F32)
            for dram, sb in ((bq, bq_sb), (bk, bk_sb), (bp, bp_sb),
                             (b1, b1_sb), (b2, b2_sb)):
                nc.sync.dma_start(out=sb[:],
                                  in_=dram.rearrange("(a p) -> p a", p=128))
            wp8_t = []
            for t in range(NP):
                w = wp8p.tile([128, 2 * DIM], F8, tag="wp8", name=f"wp8_{t}")
                nc.sync.dma_start(out=w[:], in_=ctx["Wp8"][t * 128:(t + 1) * 128, :])
                wp8_t.append(w)

            ctx.update(ones_bf=ones_bf, ones_row_bf=ones_row_bf,
                       ones_row_f=ones_row_f, eps4=eps4, eps1=eps1, negc=negc,
                       bq_sb=bq_sb, bk_sb=bk_sb, bp_sb=bp_sb, b1_sb=b1_sb,
                       b2_sb=b2_sb, wp8_t=wp8_t, xqbp=xqbp, h1p=h1p, w1p=w1p,
                       xqb_tiles=[], h1_tiles=[], w1_t=[], k_tiles=[],
                       q_tiles=[])
            av_all = avallp.tile([128, KT * QBLK], F8)
            ctx["av_all"] = av_all

            # k/q/v span QKV..attention; cos/sin + nx span phase1..QKV
            with (
                tc.tile_pool(name="ksb", bufs=KT) as ksbp,
                tc.tile_pool(name="qsb", bufs=KT) as qsbp,
                tc.tile_pool(name="vall", bufs=1) as vallp,
            ):
                ctx["ksbp"], ctx["qsbp"] = ksbp, qsbp
                ctx["v_all"] = vallp.tile([128, RT * HEADS * (HD + 1)], F8, name="v_all")
                with (
                    tc.tile_pool(name="trig", bufs=1) as trigp,
                    tc.tile_pool(name="nxall", bufs=1) as nxallp,
                ):
                    cos_sb = trigp.tile([128, T], BF16)
                    sin_sb = trigp.tile([128, T], BF16)
                    nc.sync.dma_start(out=cos_sb[:], in_=cosd[:])
                    nc.sync.dma_start(out=sin_sb[:], in_=sind[:])
                    ctx["cos_sb"], ctx["sin_sb"] = cos_sb, sin_sb
                    ctx["nx_all"] = nxallp.tile([128, KT * T], F8, name="nx_all")
                    _phase1_ln1(nc, tc, ctx)
                    _phase2_qkv(nc, tc, ctx)
                _phase3_attention(nc, tc, ctx)
            _phase4_proj(nc, tc, ctx)
            with (
                tc.tile_pool(name="nx2", bufs=KT) as nx2p,
                tc.tile_pool(name="hs", bufs=4 * KT) as hsp,
            ):
                ctx["nx2p"], ctx["hsp"] = nx2p, hsp
                nx2_tiles = []
                _phase5_ln2(nc, tc, ctx, nx2_tiles)
                _phase6_ffn(nc, tc, ctx, nx2_tiles)

    nc.compile()
    return nc


_CACHE = {}


def _pack_dr(W, scale):
    """[1024, M] f32 -> [512, 2M] fp8e4 DoubleRow pair layout, pre-scaled."""
    M = W.shape[1]
    Wp = W.reshape(NP, 2, 128, M).transpose(0, 2, 1, 3).reshape(NP * 128, 2 * M)
    return np.clip(Wp * scale, -240.0, 240.0).astype(_f8)


def _host_prep(inputs):
    g1 = np.asarray(inputs["ln1_g"], np.float32)
    b1v = np.asarray(inputs["ln1_b"], np.float32)
    g2 = np.asarray(inputs["ln2_g"], np.float32)
    b2v = np.asarray(inputs["ln2_b"], np.float32)
    W_qkv = np.asarray(inputs["W_qkv"], np.float32)
    b_qkv = np.asarray(inputs["b_qkv"], np.float32)
    W_proj = np.asarray(inputs["W_proj"], np.float32)
    b_proj = np.asarray(inputs["b_proj"], np.float32)
    W1 = np.asarray(inputs["W_ffn1"], np.float32)
    bf1 = np.asarray(inputs["b_ffn1"], np.float32)
    W2 = np.asarray(inputs["W_ffn2"], np.float32)
    bf2 = np.asarray(inputs["b_ffn2"], np.float32)

    Wf = g1[:, None] * W_qkv
    bf = b1v @ W_qkv + b_qkv
    Wq_, Wk_, Wv_ = Wf[:, :DIM], Wf[:, DIM:2 * DIM], Wf[:, 2 * DIM:]
    bq_, bk_, bv_ = bf[:DIM], bf[DIM:2 * DIM], bf[2 * DIM:]

    perm = np.empty(HD, np.int64)
    perm[0::2] = np.arange(HD // 2)
    perm[1::2] = np.arange(HD // 2) + HD // 2
    full_perm = np.concatenate([h * HD + perm for h in range(HEADS)])
    Wq_ = Wq_[:, full_perm]; bq_ = bq_[full_perm]
    Wk_ = Wk_[:, full_perm]; bk_ = bk_[full_perm]

    inv_freq = 1.0 / (ROPE_THETA ** (np.arange(0, HD, 2, dtype=np.float32) / HD))
    pos = np.arange(T, dtype=np.float32)
    ang = pos[None, :] * inv_freq[:, None]
    cosv = np.cos(ang).astype(np.float32)
    sinv = np.sin(ang).astype(np.float32)
    cos64 = np.repeat(cosv, 2, axis=0)
    sin64 = np.repeat(sinv, 2, axis=0).copy()
    sin64[0::2] *= -1.0
    cos2 = np.concatenate([cos64, cos64], axis=0).astype(_bf)
    sin2 = np.concatenate([sin64, sin64], axis=0).astype(_bf)

    bp_eff = b_proj + bv_ @ W_proj
    W1f = g2[:, None] * W1
    b1_eff = bf1 + b2v @ W1

    c = np.ascontiguousarray
    return dict(
        Wq8=c(_pack_dr(Wq_, WS)), Wk8=c(_pack_dr(Wk_, WS)),
        Wv8=c(_pack_dr(Wv_, WS)), Wp8=c(_pack_dr(W_proj, WS)),
        W1=c(W1f.astype(_bf)), W2=c(W2.astype(_bf)),
        bq=c(bq_), bk=c(bk_), bp=c(bp_eff), b1=c(b1_eff), b2=c(bf2),
        cos2=cos2, sin2=sin2,
    )


def make_in_maps(inputs):
    P = _host_prep(inputs)
    x = np.asarray(inputs["x"], np.float32)
    shared = {k: P[k] for k in ("Wq8", "Wk8", "Wv8", "Wp8", "W1", "W2",
                                "bq", "bk", "bp", "b1", "b2")}
    in_maps = []
    for c in range(NCORES):
        b = c // CPB
        qb = c % CPB
        roll = -qb * QBLK
        xTr = np.ascontiguousarray(np.roll(x[b].T, roll, axis=1))
        cosd = np.ascontiguousarray(np.roll(P["cos2"], roll, axis=1))
        sind = np.ascontiguousarray(np.roll(P["sin2"], roll, axis=1))
        in_maps.append(dict(shared, xT=xTr.astype(_bf),
                            xq=np.ascontiguousarray(xTr[:, 0:QBLK]),
                            cosd=cosd, sind=sind))
    return in_maps


def assemble_out(results):
    out = np.empty((B, T, DIM), np.float32)
    for c in range(NCORES):
        b = c // CPB
        qb = c % CPB
        out[b, qb * QBLK:(qb + 1) * QBLK, :] = results[c]["outT"].T
    return out


def get_program():
    if "nc" not in _CACHE:
        _CACHE["nc"] = _build_program()
    return _CACHE["nc"]


def kernel(**inputs):
    nc = get_program()
    in_maps = make_in_maps(inputs)
    res = run_bass_kernel_spmd(nc, in_maps, list(range(NCORES)))
    return assemble_out(res.results)


# revision 18
# speedup vs baseline: 1.2895x; 1.0150x over previous
"""Trainium2 Bass kernel for a pre-LN transformer block (MHA+RoPE, SiLU FFN).

Sharding: 8 cores; core c handles batch c//4, query block (c%4)*512..+512.
Each core redundantly computes LN1 + K/V for its whole batch (no collectives),
then attention/proj/FFN for its 512 queries. Inputs are column-rolled on the
host so every core's queries are token columns 0:512 of its xT input (SPMD
program identical across cores; RoPE tables rolled to match).

v2: fp8e4 DoubleRow matmuls for QKV/AV/proj (weights pre-scaled x64 on host,
unscaled in the PSUM-out step); q/k/es/v/av stored fp8; softmax without
max-subtraction but with a constant logit shift (C=4) so exp fits fp8 range;
x shipped bf16 (2-pass stream) with a separate f32 slice for the residual;
squares and half of RoPE on GpSimd; FFN stays bf16. RoPE's rotate-half is an
adjacent-pair swap (host-permuted q/k weight columns); LN gains/biases and the
V bias are folded into adjacent weights on the host.
"""
import sys

sys.path.insert(0, "/opt/trn_rl_repo")

import numpy as np
import ml_dtypes

import concourse.bass as bass
import concourse.mybir as mybir
from concourse import bacc
from concourse.tile import TileContext
from concourse.bass_utils import run_bass_kernel_spmd

DIM, HEADS, B, T = 1024, 16, 2, 2048
HD = DIM // HEADS          # 64
NCORES = 8
CPB = NCORES // B          # cores per batch
QBLK = T // CPB            # 512 queries per core
ROPE_THETA = 10000.0
LN_EPS = 1e-5
KT = DIM // 128            # 8 feature tiles over DIM
NCH = T // 512             # 4 column chunks over T
RT = T // 128              # 16 key-row tiles
NP = KT // 2               # 4 DoubleRow k-pairs over DIM

WS = 64.0                  # fp8 weight pre-scale
CSHIFT = 4.0               # softmax logit shift (max logit ~7.3)

F32 = mybir.dt.float32
BF16 = mybir.dt.bfloat16
F8 = mybir.dt.float8e4
AF = mybir.ActivationFunctionType
OP = mybir.AluOpType
DR = mybir.MatmulPerfMode.DoubleRow

_bf = ml_dtypes.bfloat16
_f8 = ml_dtypes.float8_e4m3


def _ln_finalize(nc, pool, ps_sum, ps_sq, eps_sb, tag):
    """From psum row-sums of x and x^2 over DIM, produce bf16 rstd and
    -mean*rstd rows ([1, 512])."""
    m_row = pool.tile([1, 512], F32, tag="lnf_m", name=f"{tag}_m")
    nc.scalar.mul(out=m_row[:], in_=ps_sum[:], mul=1.0 / DIM)
    msq = pool.tile([1, 512], F32, tag="lnf_msq", name=f"{tag}_msq")
    nc.scalar.mul(out=msq[:], in_=ps_sq[:], mul=1.0 / DIM)
    var = pool.tile([1, 512], F32, tag="lnf_var", name=f"{tag}_var")
    nc.vector.tensor_mul(out=var[:], in0=m_row[:], in1=m_row[:])
    nc.vector.tensor_sub(out=var[:], in0=msq[:], in1=var[:])
    std = pool.tile([1, 512], F32, tag="lnf_msq", name=f"{tag}_std")
    nc.scalar.activation(out=std[:], in_=var[:], func=AF.Sqrt, bias=eps_sb[:])
    rs = pool.tile([1, 512], F32, tag="lnf_rs", name=f"{tag}_rs")
    nc.vector.reciprocal_approx_fast(out=rs[:], in_=std[:])
    nm = pool.tile([1, 512], F32, tag="lnf_nm", name=f"{tag}_nm")
    nc.vector.scalar_tensor_tensor(out=nm[:], in0=m_row[:], scalar=-1.0,
                                   in1=rs[:], op0=OP.mult, op1=OP.mult)
    rs_bf = pool.tile([1, 512], BF16, tag="lnf_rsbf", name=f"{tag}_rsbf")
    nc.vector.tensor_copy(rs_bf[:], rs[:])
    nm_bf = pool.tile([1, 512], BF16, tag="lnf_nmbf", name=f"{tag}_nmbf")
    nc.vector.tensor_copy(nm_bf[:], nm[:])
    return rs_bf, nm_bf


def _phase1_ln1(nc, tc, ctx):
    """Two-pass stream of bf16 x: stats (PE ones-matmuls), batched finalize,
    broadcast, then re-stream + apply -> nx_all fp8."""
    xT, nx_all = ctx["xT"], ctx["nx_all"]
    ones_bf, ones_row_bf, eps1 = ctx["ones_bf"], ctx["ones_row_bf"], ctx["eps1"]
    with (
        tc.tile_pool(name="xbf", bufs=2) as xbfp,
        tc.tile_pool(name="xsq", bufs=2) as xsqp,
        tc.tile_pool(name="lnt", bufs=2) as lntp,
        tc.tile_pool(name="stats", bufs=1) as statp,
        tc.tile_pool(name="bcast", bufs=1) as bcastp,
        tc.tile_pool(name="ps_st", bufs=NCH, space="PSUM") as ps_stp,
    ):
        ps_sums = [ps_stp.tile([1, 512], F32, tag="ps_sum",
                               name=f"ps_sum_{n}") for n in range(NCH)]
        ps_sqs = [ps_stp.tile([1, 512], F32, tag="ps_sq",
                              name=f"ps_sq_{n}") for n in range(NCH)]
        for k in range(KT):
            xbf = xbfp.tile([128, T], BF16, tag="xbf")
            nc.sync.dma_start(out=xbf[:], in_=xT[k * 128:(k + 1) * 128, :])
            for h2 in range(2):
                hcs = slice(h2 * 1024, (h2 + 1) * 1024)
                xsq = xsqp.tile([128, 1024], BF16, tag="xsq")
                nc.gpsimd.tensor_mul(out=xsq[:], in0=xbf[:, hcs], in1=xbf[:, hcs])
                for n2 in range(2):
                    n = 2 * h2 + n2
                    cs = slice(n * 512, (n + 1) * 512)
                    nc.tensor.matmul(ps_sums[n][:], ones_bf[:], xbf[:, cs],
                                     start=(k == 0), stop=(k == KT - 1))
                    nc.tensor.matmul(ps_sqs[n][:], ones_bf[:],
                                     xsq[:, n2 * 512:(n2 + 1) * 512],
                                     start=(k == 0), stop=(k == KT - 1))

        rb_full = bcastp.tile([128, T], BF16)
        mb_full = bcastp.tile([128, T], BF16)
        for n in range(NCH):
            cs = slice(n * 512, (n + 1) * 512)
            rs_bf, nm_bf = _ln_finalize(nc, statp, ps_sums[n], ps_sqs[n],
                                        eps1, f"ln1_{n}")
            psb = ps_stp.tile([128, 512], F32, tag="ps_sum", name="ps_bc_r")
            nc.tensor.matmul(psb[:], ones_row_bf[:], rs_bf[:])
            nc.scalar.copy(out=rb_full[:, cs], in_=psb[:])
            psb2 = ps_stp.tile([128, 512], F32, tag="ps_sq", name="ps_bc_m")
            nc.tensor.matmul(psb2[:], ones_row_bf[:], nm_bf[:])
            nc.scalar.copy(out=mb_full[:, cs], in_=psb2[:])

        # pass 2: re-stream x, apply LN -> nx fp8
        for k in range(KT):
            xbf2 = xbfp.tile([128, T], BF16, tag="xbf")
            nc.sync.dma_start(out=xbf2[:], in_=xT[k * 128:(k + 1) * 128, :])
            for h2 in range(2):
                hcs = slice(h2 * 1024, (h2 + 1) * 1024)
                t1 = lntp.tile([128, 1024], BF16, tag="lnt")
                nc.vector.tensor_mul(out=t1[:], in0=xbf2[:, hcs],
                                     in1=rb_full[:, hcs])
                nc.vector.tensor_add(out=nx_all[:, k * T + h2 * 1024:
                                                k * T + (h2 + 1) * 1024],
                                     in0=t1[:], in1=mb_full[:, hcs])


def _phase2_qkv(nc, tc, ctx):
    """Q/K/V via fp8 DoubleRow; RoPE on DVE+GpSimd; V fp8 + ones column.
    Also loads W1, and builds xqb (f32 residual + proj bias)."""
    Wq8, Wk8, Wv8, W1, xq = ctx["Wq8"], ctx["Wk8"], ctx["Wv8"], ctx["W1"], ctx["xq"]
    nx_all, v_all = ctx["nx_all"], ctx["v_all"]
    cos_sb, sin_sb = ctx["cos_sb"], ctx["sin_sb"]
    bq_sb, bk_sb, bp_sb = ctx["bq_sb"], ctx["bk_sb"], ctx["bp_sb"]
    k_tiles, q_tiles = ctx["k_tiles"], ctx["q_tiles"]
    w1_t, xqb_tiles = ctx["w1_t"], ctx["xqb_tiles"]
    ksbp, qsbp, w1p, xqbp = ctx["ksbp"], ctx["qsbp"], ctx["w1p"], ctx["xqbp"]
    swap_mask = [j ^ 1 for j in range(32)]
    nxv = nx_all[:].rearrange("p (j t) -> p j t", t=T)

    with (
        tc.tile_pool(name="wqk8", bufs=NP) as wqk8p,
        tc.tile_pool(name="wv8", bufs=NP) as wv8p,
        tc.tile_pool(name="rope", bufs=2) as ropep,
        tc.tile_pool(name="ps_qk", bufs=3, space="PSUM") as ps_qkp,
        tc.tile_pool(name="ps_v", bufs=2, space="PSUM") as ps_vp,
    ):
        wk8_t, wq8_t, wv8_t = [], [], []
        for t in range(NP):
            w = wqk8p.tile([128, 2 * DIM], F8, tag="wk8", name=f"wk8_{t}")
            nc.sync.dma_start(out=w[:], in_=Wk8[t * 128:(t + 1) * 128, :])
            wk8_t.append(w)
            w = wqk8p.tile([128, 2 * DIM], F8, tag="wq8", name=f"wq8_{t}")
            nc.sync.dma_start(out=w[:], in_=Wq8[t * 128:(t + 1) * 128, :])
            wq8_t.append(w)
        for t in range(NP):
            w = wv8p.tile([128, 2 * DIM], F8, tag="wv8", name=f"wv8_{t}")
            nc.sync.dma_start(out=w[:], in_=Wv8[t * 128:(t + 1) * 128, :])
            wv8_t.append(w)

        def rope_tile(dst, raw, cols):
            n = cols.stop - cols.start
            sh = ropep.tile([128, n], BF16, tag="rope_sh", name="rope_sh")
            nc.vector.stream_shuffle(out=sh[:], in_=raw[:], mask=swap_mask)
            t2 = ropep.tile([128, n], BF16, tag="rope_t2", name="rope_t2")
            nc.gpsimd.tensor_mul(out=t2[:], in0=sh[:], in1=sin_sb[:, cols])
            t1 = ropep.tile([128, n], BF16, tag="rope_t1", name="rope_t1")
            nc.vector.tensor_mul(out=t1[:], in0=raw[:], in1=cos_sb[:, cols])
            nc.vector.tensor_add(out=dst, in0=t1[:], in1=t2[:])

        def qk_tile(w8_t, b_sb, m, dst, cs):
            ps = ps_qkp.tile([128, 512], F32, tag="ps_qk", name="ps_qk")
            ms = slice(m * 128, (m + 1) * 128)
            for t in range(NP):
                w_ap = w8_t[t][:].rearrange("p (j m) -> p j m", j=2)[:, :, ms]
                nc.tensor.matmul(ps[:], w_ap, nxv[:, 2 * t:2 * t + 2, cs],
                                 start=(t == 0), stop=(t == NP - 1), perf_mode=DR)
            raw = ropep.tile([128, 512], BF16, tag="rope_raw", name="rope_raw")
            nc.scalar.activation(out=raw[:], in_=ps[:], func=AF.Identity,
                                 scale=1.0 / WS, bias=b_sb[:, m:m + 1])
            rope_tile(dst, raw, cs)

        for m in range(KT):
            ksb = ksbp.tile([128, T], F8, tag="ksb")
            for n in range(NCH):
                cs = slice(n * 512, (n + 1) * 512)
                qk_tile(wk8_t, bk_sb, m, ksb[:, cs], cs)
            k_tiles.append(ksb)
            qsb = qsbp.tile([128, QBLK], F8, tag="qsb")
            qk_tile(wq8_t, bq_sb, m, qsb[:], slice(0, QBLK))
            q_tiles.append(qsb)

        # W1 prefetch + xq residual load (after QKV-critical DMAs)
        for k in range(KT):
            w = w1p.tile([128, 4 * DIM], BF16, tag="w1", name=f"w1_{k}")
            nc.sync.dma_start(out=w[:], in_=W1[k * 128:(k + 1) * 128, :])
            w1_t.append(w)
            xqb = xqbp.tile([128, QBLK], F32, tag="xqb", name=f"xqb_{k}")
            nc.sync.dma_start(out=xqb[:], in_=xq[k * 128:(k + 1) * 128, :])
            nc.vector.tensor_scalar_add(xqb[:], xqb[:], bp_sb[:, k:k + 1])
            xqb_tiles.append(xqb)

        # V (fp8 DoubleRow, acts-stationary), fp8 out + ones column
        vv4 = v_all[:].rearrange("p (r h c) -> p r h c", h=HEADS, c=HD + 1)
        for r in range(RT):
            rs_ = slice(r * 128, (r + 1) * 128)
            ps = ps_vp.tile([128, DIM], F32, tag="ps_v", name="ps_v")
            for vh in range(2):
                vs = slice(vh * 512, (vh + 1) * 512)
                for t in range(NP):
                    w_ap = wv8_t[t][:].rearrange("p (j m) -> p j m", j=2)[:, :, vs]
                    nc.tensor.matmul(ps[:, vs], nxv[:, 2 * t:2 * t + 2, rs_],
                                     w_ap, start=(t == 0), stop=(t == NP - 1),
                                     perf_mode=DR)
            nc.scalar.mul(out=vv4[:, r, :, 0:HD],
                          in_=ps[:].rearrange("p (h c) -> p h c", c=HD),
                          mul=1.0 / WS)
        nc.vector.memset(vv4[:, :, :, HD:HD + 1], 1.0)


def _phase3_attention(nc, tc, ctx):
    """Scores (fp8 x fp8, paired PSUM) -> exp (fp8, shifted) -> AV DoubleRow
    with ones-column denominators -> normalize -> av_all fp8."""
    v_all, av_all = ctx["v_all"], ctx["av_all"]
    k_tiles, q_tiles = ctx["k_tiles"], ctx["q_tiles"]
    ones_row_f, negc = ctx["ones_row_f"], ctx["negc"]

    with (
        tc.tile_pool(name="es", bufs=3) as esp,
        tc.tile_pool(name="rrow", bufs=2) as rrowp,
        tc.tile_pool(name="avun", bufs=2) as avunp,
        tc.tile_pool(name="ps_s", bufs=2, space="PSUM") as ps_sp,
        tc.tile_pool(name="ps_av", bufs=2, space="PSUM") as ps_avp,
        tc.tile_pool(name="ps_rb", bufs=2, space="PSUM") as ps_rbp,
    ):
        vv3 = v_all[:].rearrange("p (r hc) -> p r hc", hc=HEADS * (HD + 1))

        def scores_half(f, half, es_t):
            hsl = slice(half * HD, (half + 1) * HD)
            for rp in range(RT // 2):
                ps_s = ps_sp.tile([128, 1024], F32, tag="ps_s", name="ps_s")
                for j in range(2):
                    kt = 2 * rp + j
                    kcs = slice(kt * 128, (kt + 1) * 128)
                    nc.tensor.matmul(ps_s[:, j * 512:(j + 1) * 512],
                                     k_tiles[f][hsl, kcs], q_tiles[f][hsl, :])
                nc.scalar.activation(
                    out=es_t[half][:, rp * 1024:(rp + 1) * 1024],
                    in_=ps_s[:], func=AF.Exp,
                    scale=float(1.0 / np.sqrt(HD)), bias=negc[:])

        def av_half(f, half, es_t):
            h = 2 * f + half
            esv = es_t[half][:].rearrange("p (r n) -> p r n", n=QBLK)
            ps_av = ps_avp.tile([HD + 1, QBLK], F32, tag="ps_av",
                                name=f"ps_av_{f}_{half}")
            for rp in range(RT // 2):
                nc.tensor.matmul(
                    ps_av[:],
                    vv3[:, 2 * rp:2 * rp + 2, h * (HD + 1):(h + 1) * (HD + 1)],
                    esv[:, 2 * rp:2 * rp + 2, :],
                    start=(rp == 0), stop=(rp == RT // 2 - 1), perf_mode=DR)
            den = rrowp.tile([1, QBLK], F32, tag="den", name="den")
            nc.vector.tensor_copy(den[:], ps_av[HD:HD + 1, :])
            r_row = rrowp.tile([1, QBLK], F32, tag="r_row", name="r_row")
            nc.vector.reciprocal_approx_fast(out=r_row[:], in_=den[:])
            ps_rb = ps_rbp.tile([HD, QBLK], F32, tag="ps_rb", name="ps_rb")
            nc.tensor.matmul(ps_rb[:], ones_row_f[:], r_row[:])
            av_un = avunp.tile([HD, QBLK], BF16, tag="av_un", name="av_un")
            nc.vector.tensor_copy(av_un[:], ps_av[0:HD, :])
            nc.vector.tensor_mul(
                out=av_all[half * HD:(half + 1) * HD, f * QBLK:(f + 1) * QBLK],
                in0=av_un[:], in1=ps_rb[:])

        for f in range(HEADS // 2):
            es_t = [esp.tile([128, RT * QBLK], F8, tag=f"es{half}",
                             name=f"es_{f}_{half}") for half in range(2)]
            for half in range(2):
                scores_half(f, half, es_t)
            for half in range(2):
                av_half(f, half, es_t)


def _phase4_proj(nc, tc, ctx):
    """proj (fp8 DoubleRow) + bias + residual -> h1 f32."""
    av_all, wp8_t = ctx["av_all"], ctx["wp8_t"]
    xqb_tiles, h1_tiles, h1p = ctx["xqb_tiles"], ctx["h1_tiles"], ctx["h1p"]
    avv = av_all[:].rearrange("p (j n) -> p j n", n=QBLK)
    with tc.tile_pool(name="ps_p", bufs=3, space="PSUM") as ps_pp:
        for m in range(KT):
            ms = slice(m * 128, (m + 1) * 128)
            ps = ps_pp.tile([128, QBLK], F32, tag="ps_p", name="ps_p")
            for t in range(NP):
                w_ap = wp8_t[t][:].rearrange("p (j m) -> p j m", j=2)[:, :, ms]
                nc.tensor.matmul(ps[:], w_ap, avv[:, 2 * t:2 * t + 2, :],
                                 start=(t == 0), stop=(t == NP - 1), perf_mode=DR)
            h1 = h1p.tile([128, QBLK], F32, tag="h1", name=f"h1_{m}")
            nc.vector.scalar_tensor_tensor(
                out=h1[:], in0=ps[:], scalar=1.0 / WS,
                in1=xqb_tiles[m][:], op0=OP.mult, op1=OP.add)
            h1_tiles.append(h1)


def _phase5_ln2(nc, tc, ctx, nx2_tiles):
    """LN2: bf16 copies + gpsimd squares, PE stat sums, apply -> nx2 bf16."""
    h1_tiles = ctx["h1_tiles"]
    ones_bf, ones_row_bf, eps1 = ctx["ones_bf"], ctx["ones_row_bf"], ctx["eps1"]
    nx2p = ctx["nx2p"]
    with (
        tc.tile_pool(name="hbf", bufs=2) as hbfp,
        tc.tile_pool(name="hsq", bufs=2) as hsqp,
        tc.tile_pool(name="ln2t", bufs=2) as ln2tp,
        tc.tile_pool(name="stats2", bufs=1) as stat2p,
        tc.tile_pool(name="bcast2", bufs=1) as bcast2p,
        tc.tile_pool(name="ps_st2", bufs=2, space="PSUM") as ps_st2p,
    ):
        ps_sum = ps_st2p.tile([1, 512], F32, tag="ps_sum2", name="ps_sum2")
        ps_sq = ps_st2p.tile([1, 512], F32, tag="ps_sq2", name="ps_sq2")
        for k in range(KT):
            hbf = hbfp.tile([128, QBLK], BF16, tag="hbf")
            nc.vector.tensor_copy(hbf[:], h1_tiles[k][:])
            hsq = hsqp.tile([128, QBLK], BF16, tag="hsq")
            nc.gpsimd.tensor_mul(out=hsq[:], in0=hbf[:], in1=hbf[:])
            nc.tensor.matmul(ps_sum[:], ones_bf[:], hbf[:],
                             start=(k == 0), stop=(k == KT - 1))
            nc.tensor.matmul(ps_sq[:], ones_bf[:], hsq[:],
                             start=(k == 0), stop=(k == KT - 1))
        rs_bf2, nm_bf2 = _ln_finalize(nc, stat2p, ps_sum, ps_sq, eps1, "ln2")
        rb2 = bcast2p.tile([128, QBLK], BF16)
        mb2 = bcast2p.tile([128, QBLK], BF16)
        psb = ps_st2p.tile([128, 512], F32, tag="ps_sum2", name="ps_bc2r")
        nc.tensor.matmul(psb[:], ones_row_bf[:], rs_bf2[:])
        nc.scalar.copy(out=rb2[:], in_=psb[:])
        psb2 = ps_st2p.tile([128, 512], F32, tag="ps_sq2", name="ps_bc2m")
        nc.tensor.matmul(psb2[:], ones_row_bf[:], nm_bf2[:])
        nc.scalar.copy(out=mb2[:], in_=psb2[:])
        for k in range(KT):
            t1 = ln2tp.tile([128, QBLK], BF16, tag="ln2t", name="ln2t")
            nc.vector.tensor_mul(out=t1[:], in0=h1_tiles[k][:], in1=rb2[:])
            nx2 = nx2p.tile([128, QBLK], BF16, tag="nx2")
            nc.vector.tensor_add(out=nx2[:], in0=t1[:], in1=mb2[:])
            nx2_tiles.append(nx2)


def _phase6_ffn(nc, tc, ctx, nx2_tiles):
    """FFN1 (bf16, resident W1) + SiLU; FFN2 (bf16, streamed W2) + residual."""
    W2, outT = ctx["W2"], ctx["outT"]
    w1_t, h1_tiles = ctx["w1_t"], ctx["h1_tiles"]
    b1_sb, b2_sb, hsp = ctx["b1_sb"], ctx["b2_sb"], ctx["hsp"]
    hs_tiles = []
    with tc.tile_pool(name="ps_f", bufs=3, space="PSUM") as ps_fp:
        for m in range(4 * KT):
            ms = slice(m * 128, (m + 1) * 128)
            ps = ps_fp.tile([128, QBLK], F32, tag="ps_f", name="ps_f")
            for k in range(KT):
                nc.tensor.matmul(ps[:], w1_t[k][:, ms], nx2_tiles[k][:],
                                 start=(k == 0), stop=(k == KT - 1))
            hs = hsp.tile([128, QBLK], BF16, tag="hs", name="hs")
            nc.scalar.activation(out=hs[:], in_=ps[:], func=AF.Silu,
                                 bias=b1_sb[:, m:m + 1])
            hs_tiles.append(hs)

    with (
        tc.tile_pool(name="w2", bufs=10) as w2p,
        tc.tile_pool(name="osb", bufs=2) as osbp,
        tc.tile_pool(name="ps_o", bufs=KT, space="PSUM") as ps_op,
    ):
        ps_o = [ps_op.tile([128, QBLK], F32, tag="ps_o", name=f"ps_o_{i}")
                for i in range(KT)]
        for k in range(4 * KT):
            w2 = w2p.tile([128, DIM], BF16, tag="w2", name="w2")
            nc.sync.dma_start(out=w2[:], in_=W2[k * 128:(k + 1) * 128, :])
            for m in range(KT):
                nc.tensor.matmul(ps_o[m][:], w2[:, m * 128:(m + 1) * 128],
                                 hs_tiles[k][:],
                                 start=(k == 0), stop=(k == 4 * KT - 1))
        for m in range(KT):
            osb = osbp.tile([128, QBLK], F32, tag="osb", name="osb")
            nc.vector.scalar_tensor_tensor(
                out=osb[:], in0=ps_o[m][:], scalar=b2_sb[:, m:m + 1],
                in1=h1_tiles[m][:], op0=OP.add, op1=OP.add)
            nc.sync.dma_start(out=outT[m * 128:(m + 1) * 128, :], in_=osb[:])


def _build_program():
    nc = bacc.Bacc("TRN2", target_bir_lowering=False, debug=False,
                   num_devices=NCORES)

    ctx = {}
    ctx["xT"] = nc.declare_dram_parameter("xT", [DIM, T], BF16, isOutput=False)
    ctx["xq"] = nc.declare_dram_parameter("xq", [DIM, QBLK], F32, isOutput=False)
    cosd = nc.declare_dram_parameter("cosd", [128, T], BF16, isOutput=False)
    sind = nc.declare_dram_parameter("sind", [128, T], BF16, isOutput=False)
    for nm in ("Wq8", "Wk8", "Wv8", "Wp8"):
        ctx[nm] = nc.declare_dram_parameter(nm, [NP * 128, 2 * DIM], F8,
                                            isOutput=False)
    ctx["W1"] = nc.declare_dram_parameter("W1", [DIM, 4 * DIM], BF16,
                                          isOutput=False)
    ctx["W2"] = nc.declare_dram_parameter("W2", [4 * DIM, DIM], BF16,
                                          isOutput=False)
    bq = nc.declare_dram_parameter("bq", [DIM], F32, isOutput=False)
    bk = nc.declare_dram_parameter("bk", [DIM], F32, isOutput=False)
    bp = nc.declare_dram_parameter("bp", [DIM], F32, isOutput=False)
    b1 = nc.declare_dram_parameter("b1", [4 * DIM], F32, isOutput=False)
    b2 = nc.declare_dram_parameter("b2", [DIM], F32, isOutput=False)
    ctx["outT"] = nc.declare_dram_parameter("outT", [DIM, QBLK], F32,
                                            isOutput=True)

    with TileContext(nc) as tc:
        with (
            tc.tile_pool(name="consts", bufs=1) as consts,
            tc.tile_pool(name="xqb", bufs=KT) as xqbp,
            tc.tile_pool(name="h1", bufs=KT) as h1p,
            tc.tile_pool(name="avall", bufs=1) as avallp,
            tc.tile_pool(name="wp8", bufs=NP) as wp8p,
            tc.tile_pool(name="w1", bufs=KT) as w1p,
        ):
            ones_bf = consts.tile([128, 1], BF16)
            nc.vector.memset(ones_bf[:], 1.0)
            ones_row_bf = consts.tile([1, 128], BF16)
            nc.vector.memset(ones_row_bf[:], 1.0)
            ones_row_f = consts.tile([1, HD], F32)
            nc.vector.memset(ones_row_f[:], 1.0)
            eps4 = consts.tile([NCH, 1], F32)
            nc.vector.memset(eps4[:], LN_EPS)
            eps1 = consts.tile([1, 1], F32)
            nc.vector.memset(eps1[:], LN_EPS)
            negc = consts.tile([128, 1], F32)
            nc.vector.memset(negc[:], -CSHIFT)
            bq_sb = consts.tile([128, KT], F32)
            bk_sb = consts.tile([128, KT], F32)
            bp_sb = consts.tile([128, KT], F32)
            b1_sb = consts.tile([128, 4 * KT], F32)
            b2_sb = consts.tile([128, KT], F32)
            for dram, sb in ((bq, bq_sb), (bk, bk_sb), (bp, bp_sb),
                             (b1, b1_sb), (b2, b2_sb)):
                nc.sync.dma_start(out=sb[:],
                                  in_=dram.rearrange("(a p) -> p a", p=128))
            wp8_t = []
            for t in range(NP):
                w = wp8p.tile([128, 2 * DIM], F8, tag="wp8", name=f"wp8_{t}")
                nc.sync.dma_start(out=w[:], in_=ctx["Wp8"][t * 128:(t + 1) * 128, :])
                wp8_t.append(w)

            ctx.update(ones_bf=ones_bf, ones_row_bf=ones_row_bf,
                       ones_row_f=ones_row_f, eps4=eps4, eps1=eps1, negc=negc,
                       bq_sb=bq_sb, bk_sb=bk_sb, bp_sb=bp_sb, b1_sb=b1_sb,
                       b2_sb=b2_sb, wp8_t=wp8_t, xqbp=xqbp, h1p=h1p, w1p=w1p,
                       xqb_tiles=[], h1_tiles=[], w1_t=[], k_tiles=[],
                       q_tiles=[])
            av_all = avallp.tile([128, KT * QBLK], F8)
            ctx["av_all"] = av_all

            # k/q/v span QKV..attention; cos/sin + nx span phase1..QKV
            with (
                tc.tile_pool(name="ksb", bufs=KT) as ksbp,
                tc.tile_pool(name="qsb", bufs=KT) as qsbp,
                tc.tile_pool(name="vall", bufs=1) as vallp,
            ):
                ctx["ksbp"], ctx["qsbp"] = ksbp, qsbp
                ctx["v_all"] = vallp.tile([128, RT * HEADS * (HD + 1)], F8, name="v_all")
                with (
                    tc.tile_pool(name="trig", bufs=1) as trigp,
                    tc.tile_pool(name="nxall", bufs=1) as nxallp,
                ):
                    cos_sb = trigp.tile([128, T], BF16)
                    sin_sb = trigp.tile([128, T], BF16)
                    nc.sync.dma_start(out=cos_sb[:], in_=cosd[:])
                    nc.sync.dma_start(out=sin_sb[:], in_=sind[:])
                    ctx["cos_sb"], ctx["sin_sb"] = cos_sb, sin_sb
                    ctx["nx_all"] = nxallp.tile([128, KT * T], F8, name="nx_all")
                    _phase1_ln1(nc, tc, ctx)
                    _phase2_qkv(nc, tc, ctx)
                _phase3_attention(nc, tc, ctx)
            _phase4_proj(nc, tc, ctx)
            with (
                tc.tile_pool(name="nx2", bufs=KT) as nx2p,
                tc.tile_pool(name="hs", bufs=4 * KT) as hsp,
            ):
                ctx["nx2p"], ctx["hsp"] = nx2p, hsp
                nx2_tiles = []
                _phase5_ln2(nc, tc, ctx, nx2_tiles)
                _phase6_ffn(nc, tc, ctx, nx2_tiles)

    nc.compile()
    return nc


_CACHE = {}


def _pack_dr(W, scale):
    """[1024, M] f32 -> [512, 2M] fp8e4 DoubleRow pair layout, pre-scaled."""
    M = W.shape[1]
    Wp = W.reshape(NP, 2, 128, M).transpose(0, 2, 1, 3).reshape(NP * 128, 2 * M)
    return np.clip(Wp * scale, -240.0, 240.0).astype(_f8)


def _host_prep(inputs):
    g1 = np.asarray(inputs["ln1_g"], np.float32)
    b1v = np.asarray(inputs["ln1_b"], np.float32)
    g2 = np.asarray(inputs["ln2_g"], np.float32)
    b2v = np.asarray(inputs["ln2_b"], np.float32)
    W_qkv = np.asarray(inputs["W_qkv"], np.float32)
    b_qkv = np.asarray(inputs["b_qkv"], np.float32)
    W_proj = np.asarray(inputs["W_proj"], np.float32)
    b_proj = np.asarray(inputs["b_proj"], np.float32)
    W1 = np.asarray(inputs["W_ffn1"], np.float32)
    bf1 = np.asarray(inputs["b_ffn1"], np.float32)
    W2 = np.asarray(inputs["W_ffn2"], np.float32)
    bf2 = np.asarray(inputs["b_ffn2"], np.float32)

    Wf = g1[:, None] * W_qkv
    bf = b1v @ W_qkv + b_qkv
    Wq_, Wk_, Wv_ = Wf[:, :DIM], Wf[:, DIM:2 * DIM], Wf[:, 2 * DIM:]
    bq_, bk_, bv_ = bf[:DIM], bf[DIM:2 * DIM], bf[2 * DIM:]

    perm = np.empty(HD, np.int64)
    perm[0::2] = np.arange(HD // 2)
    perm[1::2] = np.arange(HD // 2) + HD // 2
    full_perm = np.concatenate([h * HD + perm for h in range(HEADS)])
    Wq_ = Wq_[:, full_perm]; bq_ = bq_[full_perm]
    Wk_ = Wk_[:, full_perm]; bk_ = bk_[full_perm]

    inv_freq = 1.0 / (ROPE_THETA ** (np.arange(0, HD, 2, dtype=np.float32) / HD))
    pos = np.arange(T, dtype=np.float32)
    ang = pos[None, :] * inv_freq[:, None]
    cosv = np.cos(ang).astype(np.float32)
    sinv = np.sin(ang).astype(np.float32)
    cos64 = np.repeat(cosv, 2, axis=0)
    sin64 = np.repeat(sinv, 2, axis=0).copy()
    sin64[0::2] *= -1.0
    cos2 = np.concatenate([cos64, cos64], axis=0).astype(_bf)
    sin2 = np.concatenate([sin64, sin64], axis=0).astype(_bf)

    bp_eff = b_proj + bv_ @ W_proj
    W1f = g2[:, None] * W1
    b1_eff = bf1 + b2v @ W1

    c = np.ascontiguousarray
    return dict(
        Wq8=c(_pack_dr(Wq_, WS)), Wk8=c(_pack_dr(Wk_, WS)),
        Wv8=c(_pack_dr(Wv_, WS)), Wp8=c(_pack_dr(W_proj, WS)),
        W1=c(W1f.astype(_bf)), W2=c(W2.astype(_bf)),
        bq=c(bq_), bk=c(bk_), bp=c(bp_eff), b1=c(b1_eff), b2=c(bf2),
        cos2=cos2, sin2=sin2,
    )


def make_in_maps(inputs):
    P = _host_prep(inputs)
    x = np.asarray(inputs["x"], np.float32)
    shared = {k: P[k] for k in ("Wq8", "Wk8", "Wv8", "Wp8", "W1", "W2",
                                "bq", "bk", "bp", "b1", "b2")}
    in_maps = []
    for c in range(NCORES):
        b = c // CPB
        qb = c % CPB
        roll = -qb * QBLK
        xTr = np.ascontiguousarray(np.roll(x[b].T, roll, axis=1))
        cosd = np.ascontiguousarray(np.roll(P["cos2"], roll, axis=1))
        sind = np.ascontiguousarray(np.roll(P["sin2"], roll, axis=1))
        in_maps.append(dict(shared, xT=xTr.astype(_bf),
                            xq=np.ascontiguousarray(xTr[:, 0:QBLK]),
                            cosd=cosd, sind=sind))
    return in_maps


def assemble_out(results):
    out = np.empty((B, T, DIM), np.float32)
    for c in range(NCORES):
        b = c // CPB
        qb = c % CPB
        out[b, qb * QBLK:(qb + 1) * QBLK, :] = results[c]["outT"].T
    return out


def get_program():
    if "nc" not in _CACHE:
        _CACHE["nc"] = _build_program()
    return _CACHE["nc"]


def kernel(**inputs):
    nc = get_program()
    in_maps = make_in_maps(inputs)
    res = run_bass_kernel_spmd(nc, in_maps, list(range(NCORES)))
    return assemble_out(res.results)


# revision 19
# speedup vs baseline: 1.3377x; 1.0374x over previous
"""Trainium2 Bass kernel for a pre-LN transformer block (MHA+RoPE, SiLU FFN).

Sharding: 8 cores; core c handles batch c//4, query block (c%4)*512..+512.
Each core redundantly computes LN1 + K/V for its whole batch (no collectives),
then attention/proj/FFN for its 512 queries. Inputs are column-rolled on the
host so every core's queries are token columns 0:512 of its xT input (SPMD
program identical across cores; RoPE tables rolled to match).

v2: fp8e4 DoubleRow matmuls for QKV/AV/proj (weights pre-scaled x64 on host,
unscaled in the PSUM-out step); q/k/es/v/av stored fp8; softmax without
max-subtraction but with a constant logit shift (C=4) so exp fits fp8 range;
x shipped bf16 (2-pass stream) with a separate f32 slice for the residual;
squares and half of RoPE on GpSimd; FFN stays bf16. RoPE's rotate-half is an
adjacent-pair swap (host-permuted q/k weight columns); LN gains/biases and the
V bias are folded into adjacent weights on the host.
"""
import sys

sys.path.insert(0, "/opt/trn_rl_repo")

import numpy as np
import ml_dtypes

import concourse.bass as bass
import concourse.mybir as mybir
from concourse import bacc
from concourse.tile import TileContext
from concourse.bass_utils import run_bass_kernel_spmd

DIM, HEADS, B, T = 1024, 16, 2, 2048
HD = DIM // HEADS          # 64
NCORES = 8
CPB = NCORES // B          # cores per batch
QBLK = T // CPB            # 512 queries per core
ROPE_THETA = 10000.0
LN_EPS = 1e-5
KT = DIM // 128            # 8 feature tiles over DIM
NCH = T // 512             # 4 column chunks over T
RT = T // 128              # 16 key-row tiles
NP = KT // 2               # 4 DoubleRow k-pairs over DIM

WS = 64.0                  # fp8 weight pre-scale
CSHIFT = 4.0               # softmax logit shift (max logit ~7.3)

F32 = mybir.dt.float32
BF16 = mybir.dt.bfloat16
F8 = mybir.dt.float8e4
AF = mybir.ActivationFunctionType
OP = mybir.AluOpType
DR = mybir.MatmulPerfMode.DoubleRow

_bf = ml_dtypes.bfloat16
_f8 = ml_dtypes.float8_e4m3


def _ln_finalize(nc, pool, ps_sum, ps_sq, eps_sb, tag):
    """From psum row-sums of x and x^2 over DIM, produce bf16 rstd and
    -mean*rstd rows ([1, 512])."""
    m_row = pool.tile([1, 512], F32, tag="lnf_m", name=f"{tag}_m")
    nc.scalar.mul(out=m_row[:], in_=ps_sum[:], mul=1.0 / DIM)
    msq = pool.tile([1, 512], F32, tag="lnf_msq", name=f"{tag}_msq")
    nc.scalar.mul(out=msq[:], in_=ps_sq[:], mul=1.0 / DIM)
    var = pool.tile([1, 512], F32, tag="lnf_var", name=f"{tag}_var")
    nc.vector.tensor_mul(out=var[:], in0=m_row[:], in1=m_row[:])
    nc.vector.tensor_sub(out=var[:], in0=msq[:], in1=var[:])
    std = pool.tile([1, 512], F32, tag="lnf_msq", name=f"{tag}_std")
    nc.scalar.activation(out=std[:], in_=var[:], func=AF.Sqrt, bias=eps_sb[:])
    rs = pool.tile([1, 512], F32, tag="lnf_rs", name=f"{tag}_rs")
    nc.vector.reciprocal_approx_fast(out=rs[:], in_=std[:])
    nm = pool.tile([1, 512], F32, tag="lnf_nm", name=f"{tag}_nm")
    nc.vector.scalar_tensor_tensor(out=nm[:], in0=m_row[:], scalar=-1.0,
                                   in1=rs[:], op0=OP.mult, op1=OP.mult)
    rs_bf = pool.tile([1, 512], BF16, tag="lnf_rsbf", name=f"{tag}_rsbf")
    nc.vector.tensor_copy(rs_bf[:], rs[:])
    nm_bf = pool.tile([1, 512], BF16, tag="lnf_nmbf", name=f"{tag}_nmbf")
    nc.vector.tensor_copy(nm_bf[:], nm[:])
    return rs_bf, nm_bf


def _phase1_ln1(nc, tc, ctx):
    """Two-pass stream of bf16 x: stats (PE ones-matmuls), batched finalize,
    broadcast, then re-stream + apply -> nx_all fp8."""
    xT, nx_all = ctx["xT"], ctx["nx_all"]
    ones_bf, ones_row_bf, eps1 = ctx["ones_bf"], ctx["ones_row_bf"], ctx["eps1"]
    with (
        tc.tile_pool(name="xbf", bufs=2) as xbfp,
        tc.tile_pool(name="xsq", bufs=2) as xsqp,
        tc.tile_pool(name="lnt", bufs=2) as lntp,
        tc.tile_pool(name="stats", bufs=1) as statp,
        tc.tile_pool(name="bcast", bufs=1) as bcastp,
        tc.tile_pool(name="ps_st", bufs=NCH, space="PSUM") as ps_stp,
    ):
        ps_sums = [ps_stp.tile([1, 512], F32, tag="ps_sum",
                               name=f"ps_sum_{n}") for n in range(NCH)]
        ps_sqs = [ps_stp.tile([1, 512], F32, tag="ps_sq",
                              name=f"ps_sq_{n}") for n in range(NCH)]
        for k in range(KT):
            xbf = xbfp.tile([128, T], BF16, tag="xbf")
            nc.sync.dma_start(out=xbf[:], in_=xT[k * 128:(k + 1) * 128, :])
            for h2 in range(2):
                hcs = slice(h2 * 1024, (h2 + 1) * 1024)
                xsq = xsqp.tile([128, 1024], BF16, tag="xsq")
                nc.vector.tensor_mul(out=xsq[:], in0=xbf[:, hcs], in1=xbf[:, hcs])
                for n2 in range(2):
                    n = 2 * h2 + n2
                    cs = slice(n * 512, (n + 1) * 512)
                    nc.tensor.matmul(ps_sums[n][:], ones_bf[:], xbf[:, cs],
                                     start=(k == 0), stop=(k == KT - 1))
                    nc.tensor.matmul(ps_sqs[n][:], ones_bf[:],
                                     xsq[:, n2 * 512:(n2 + 1) * 512],
                                     start=(k == 0), stop=(k == KT - 1))

        rb_full = bcastp.tile([128, T], BF16)
        mb_full = bcastp.tile([128, T], BF16)
        for n in range(NCH):
            cs = slice(n * 512, (n + 1) * 512)
            rs_bf, nm_bf = _ln_finalize(nc, statp, ps_sums[n], ps_sqs[n],
                                        eps1, f"ln1_{n}")
            psb = ps_stp.tile([128, 512], F32, tag="ps_sum", name="ps_bc_r")
            nc.tensor.matmul(psb[:], ones_row_bf[:], rs_bf[:])
            nc.scalar.copy(out=rb_full[:, cs], in_=psb[:])
            psb2 = ps_stp.tile([128, 512], F32, tag="ps_sq", name="ps_bc_m")
            nc.tensor.matmul(psb2[:], ones_row_bf[:], nm_bf[:])
            nc.scalar.copy(out=mb_full[:, cs], in_=psb2[:])

        # pass 2: re-stream x, apply LN -> nx fp8
        for k in range(KT):
            xbf2 = xbfp.tile([128, T], BF16, tag="xbf")
            nc.sync.dma_start(out=xbf2[:], in_=xT[k * 128:(k + 1) * 128, :])
            for h2 in range(2):
                hcs = slice(h2 * 1024, (h2 + 1) * 1024)
                t1 = lntp.tile([128, 1024], BF16, tag="lnt")
                nc.vector.tensor_mul(out=t1[:], in0=xbf2[:, hcs],
                                     in1=rb_full[:, hcs])
                nc.vector.tensor_add(out=nx_all[:, k * T + h2 * 1024:
                                                k * T + (h2 + 1) * 1024],
                                     in0=t1[:], in1=mb_full[:, hcs])


def _phase2_qkv(nc, tc, ctx):
    """Q/K/V via fp8 DoubleRow; RoPE on DVE+GpSimd; V fp8 + ones column.
    Also loads W1, and builds xqb (f32 residual + proj bias)."""
    Wq8, Wk8, Wv8, W1, xq = ctx["Wq8"], ctx["Wk8"], ctx["Wv8"], ctx["W1"], ctx["xq"]
    nx_all, v_all = ctx["nx_all"], ctx["v_all"]
    cos_sb, sin_sb = ctx["cos_sb"], ctx["sin_sb"]
    bq_sb, bk_sb, bp_sb = ctx["bq_sb"], ctx["bk_sb"], ctx["bp_sb"]
    k_tiles, q_tiles = ctx["k_tiles"], ctx["q_tiles"]
    w1_t, xqb_tiles = ctx["w1_t"], ctx["xqb_tiles"]
    ksbp, qsbp, w1p, xqbp = ctx["ksbp"], ctx["qsbp"], ctx["w1p"], ctx["xqbp"]
    swap_mask = [j ^ 1 for j in range(32)]
    nxv = nx_all[:].rearrange("p (j t) -> p j t", t=T)

    with (
        tc.tile_pool(name="wqk8", bufs=NP) as wqk8p,
        tc.tile_pool(name="wv8", bufs=NP) as wv8p,
        tc.tile_pool(name="rope", bufs=2) as ropep,
        tc.tile_pool(name="ps_qk", bufs=3, space="PSUM") as ps_qkp,
        tc.tile_pool(name="ps_v", bufs=2, space="PSUM") as ps_vp,
    ):
        wk8_t, wq8_t, wv8_t = [], [], []
        for t in range(NP):
            w = wqk8p.tile([128, 2 * DIM], F8, tag="wk8", name=f"wk8_{t}")
            nc.sync.dma_start(out=w[:], in_=Wk8[t * 128:(t + 1) * 128, :])
            wk8_t.append(w)
            w = wqk8p.tile([128, 2 * DIM], F8, tag="wq8", name=f"wq8_{t}")
            nc.sync.dma_start(out=w[:], in_=Wq8[t * 128:(t + 1) * 128, :])
            wq8_t.append(w)
        for t in range(NP):
            w = wv8p.tile([128, 2 * DIM], F8, tag="wv8", name=f"wv8_{t}")
            nc.sync.dma_start(out=w[:], in_=Wv8[t * 128:(t + 1) * 128, :])
            wv8_t.append(w)

        rope_ctr = [0]

        def rope_tile(dst, raw, cols):
            n = cols.stop - cols.start
            sh = ropep.tile([128, n], BF16, tag="rope_sh", name="rope_sh")
            nc.vector.stream_shuffle(out=sh[:], in_=raw[:], mask=swap_mask)
            t2 = ropep.tile([128, n], BF16, tag="rope_t2", name="rope_t2")
            nc.gpsimd.tensor_mul(out=t2[:], in0=sh[:], in1=sin_sb[:, cols])
            t1 = ropep.tile([128, n], BF16, tag="rope_t1", name="rope_t1")
            nc.vector.tensor_mul(out=t1[:], in0=raw[:], in1=cos_sb[:, cols])
            rope_ctr[0] += 1
            eng = nc.vector if rope_ctr[0] % 2 else nc.gpsimd
            eng.tensor_add(out=dst, in0=t1[:], in1=t2[:])

        def qk_tile(w8_t, b_sb, m, dst, cs):
            ps = ps_qkp.tile([128, 512], F32, tag="ps_qk", name="ps_qk")
            ms = slice(m * 128, (m + 1) * 128)
            for t in range(NP):
                w_ap = w8_t[t][:].rearrange("p (j m) -> p j m", j=2)[:, :, ms]
                nc.tensor.matmul(ps[:], w_ap, nxv[:, 2 * t:2 * t + 2, cs],
                                 start=(t == 0), stop=(t == NP - 1), perf_mode=DR)
            raw = ropep.tile([128, 512], BF16, tag="rope_raw", name="rope_raw")
            nc.scalar.activation(out=raw[:], in_=ps[:], func=AF.Identity,
                                 scale=1.0 / WS, bias=b_sb[:, m:m + 1])
            rope_tile(dst, raw, cs)

        for m in range(KT):
            ksb = ksbp.tile([128, T], F8, tag="ksb")
            for n in range(NCH):
                cs = slice(n * 512, (n + 1) * 512)
                qk_tile(wk8_t, bk_sb, m, ksb[:, cs], cs)
            k_tiles.append(ksb)
            qsb = qsbp.tile([128, QBLK], F8, tag="qsb")
            qk_tile(wq8_t, bq_sb, m, qsb[:], slice(0, QBLK))
            q_tiles.append(qsb)

        # W1 prefetch + xq residual load (after QKV-critical DMAs)
        for k in range(KT):
            w = w1p.tile([128, 4 * DIM], BF16, tag="w1", name=f"w1_{k}")
            nc.sync.dma_start(out=w[:], in_=W1[k * 128:(k + 1) * 128, :])
            w1_t.append(w)
            xqb = xqbp.tile([128, QBLK], F32, tag="xqb", name=f"xqb_{k}")
            nc.sync.dma_start(out=xqb[:], in_=xq[k * 128:(k + 1) * 128, :])
            nc.vector.tensor_scalar_add(xqb[:], xqb[:], bp_sb[:, k:k + 1])
            xqb_tiles.append(xqb)

        # V (fp8 DoubleRow, acts-stationary), fp8 out + ones column
        vv4 = v_all[:].rearrange("p (r h c) -> p r h c", h=HEADS, c=HD + 1)
        for r in range(RT):
            rs_ = slice(r * 128, (r + 1) * 128)
            ps = ps_vp.tile([128, DIM], F32, tag="ps_v", name="ps_v")
            for vh in range(2):
                vs = slice(vh * 512, (vh + 1) * 512)
                for t in range(NP):
                    w_ap = wv8_t[t][:].rearrange("p (j m) -> p j m", j=2)[:, :, vs]
                    nc.tensor.matmul(ps[:, vs], nxv[:, 2 * t:2 * t + 2, rs_],
                                     w_ap, start=(t == 0), stop=(t == NP - 1),
                                     perf_mode=DR)
            nc.scalar.mul(out=vv4[:, r, :, 0:HD],
                          in_=ps[:].rearrange("p (h c) -> p h c", c=HD),
                          mul=1.0 / WS)
        nc.vector.memset(vv4[:, :, :, HD:HD + 1], 1.0)


def _phase3_attention(nc, tc, ctx):
    """Scores (fp8 x fp8, paired PSUM) -> exp (fp8, shifted) -> AV DoubleRow
    with ones-column denominators -> normalize -> av_all fp8."""
    v_all, av_all = ctx["v_all"], ctx["av_all"]
    k_tiles, q_tiles = ctx["k_tiles"], ctx["q_tiles"]
    ones_row_f, negc = ctx["ones_row_f"], ctx["negc"]

    with (
        tc.tile_pool(name="es", bufs=3) as esp,
        tc.tile_pool(name="rrow", bufs=2) as rrowp,
        tc.tile_pool(name="avun", bufs=2) as avunp,
        tc.tile_pool(name="ps_s", bufs=3, space="PSUM") as ps_sp,
        tc.tile_pool(name="ps_av", bufs=1, space="PSUM") as ps_avp,
        tc.tile_pool(name="ps_rb", bufs=1, space="PSUM") as ps_rbp,
    ):
        vv3 = v_all[:].rearrange("p (r hc) -> p r hc", hc=HEADS * (HD + 1))

        def scores_half(f, half, es_t):
            hsl = slice(half * HD, (half + 1) * HD)
            for rp in range(RT // 2):
                ps_s = ps_sp.tile([128, 1024], F32, tag="ps_s", name="ps_s")
                for j in range(2):
                    kt = 2 * rp + j
                    kcs = slice(kt * 128, (kt + 1) * 128)
                    nc.tensor.matmul(ps_s[:, j * 512:(j + 1) * 512],
                                     k_tiles[f][hsl, kcs], q_tiles[f][hsl, :])
                nc.scalar.activation(
                    out=es_t[half][:, rp * 1024:(rp + 1) * 1024],
                    in_=ps_s[:], func=AF.Exp,
                    scale=float(1.0 / np.sqrt(HD)), bias=negc[:])

        def av_half(f, half, es_t):
            h = 2 * f + half
            esv = es_t[half][:].rearrange("p (r n) -> p r n", n=QBLK)
            ps_av = ps_avp.tile([HD + 1, QBLK], F32, tag="ps_av",
                                name=f"ps_av_{f}_{half}")
            for rp in range(RT // 2):
                nc.tensor.matmul(
                    ps_av[:],
                    vv3[:, 2 * rp:2 * rp + 2, h * (HD + 1):(h + 1) * (HD + 1)],
                    esv[:, 2 * rp:2 * rp + 2, :],
                    start=(rp == 0), stop=(rp == RT // 2 - 1), perf_mode=DR)
            den = rrowp.tile([1, QBLK], F32, tag="den", name="den")
            nc.vector.tensor_copy(den[:], ps_av[HD:HD + 1, :])
            r_row = rrowp.tile([1, QBLK], F32, tag="r_row", name="r_row")
            nc.vector.reciprocal_approx_fast(out=r_row[:], in_=den[:])
            ps_rb = ps_rbp.tile([HD, QBLK], F32, tag="ps_rb", name="ps_rb")
            nc.tensor.matmul(ps_rb[:], ones_row_f[:], r_row[:])
            av_un = avunp.tile([HD, QBLK], BF16, tag="av_un", name="av_un")
            nc.vector.tensor_copy(av_un[:], ps_av[0:HD, :])
            nc.vector.tensor_mul(
                out=av_all[half * HD:(half + 1) * HD, f * QBLK:(f + 1) * QBLK],
                in0=av_un[:], in1=ps_rb[:])

        for f in range(HEADS // 2):
            es_t = [esp.tile([128, RT * QBLK], F8, tag=f"es{half}",
                             name=f"es_{f}_{half}") for half in range(2)]
            for half in range(2):
                scores_half(f, half, es_t)
            for half in range(2):
                av_half(f, half, es_t)


def _phase4_proj(nc, tc, ctx):
    """proj (fp8 DoubleRow) + bias + residual -> h1 f32."""
    av_all, wp8_t = ctx["av_all"], ctx["wp8_t"]
    xqb_tiles, h1_tiles, h1p = ctx["xqb_tiles"], ctx["h1_tiles"], ctx["h1p"]
    avv = av_all[:].rearrange("p (j n) -> p j n", n=QBLK)
    with tc.tile_pool(name="ps_p", bufs=3, space="PSUM") as ps_pp:
        for m in range(KT):
            ms = slice(m * 128, (m + 1) * 128)
            ps = ps_pp.tile([128, QBLK], F32, tag="ps_p", name="ps_p")
            for t in range(NP):
                w_ap = wp8_t[t][:].rearrange("p (j m) -> p j m", j=2)[:, :, ms]
                nc.tensor.matmul(ps[:], w_ap, avv[:, 2 * t:2 * t + 2, :],
                                 start=(t == 0), stop=(t == NP - 1), perf_mode=DR)
            h1 = h1p.tile([128, QBLK], F32, tag="h1", name=f"h1_{m}")
            nc.vector.scalar_tensor_tensor(
                out=h1[:], in0=ps[:], scalar=1.0 / WS,
                in1=xqb_tiles[m][:], op0=OP.mult, op1=OP.add)
            h1_tiles.append(h1)


def _phase5_ln2(nc, tc, ctx, nx2_tiles):
    """LN2: bf16 copies + gpsimd squares, PE stat sums, apply -> nx2 bf16."""
    h1_tiles = ctx["h1_tiles"]
    ones_bf, ones_row_bf, eps1 = ctx["ones_bf"], ctx["ones_row_bf"], ctx["eps1"]
    nx2p = ctx["nx2p"]
    with (
        tc.tile_pool(name="hbf", bufs=2) as hbfp,
        tc.tile_pool(name="hsq", bufs=2) as hsqp,
        tc.tile_pool(name="ln2t", bufs=2) as ln2tp,
        tc.tile_pool(name="stats2", bufs=1) as stat2p,
        tc.tile_pool(name="bcast2", bufs=1) as bcast2p,
        tc.tile_pool(name="ps_st2", bufs=2, space="PSUM") as ps_st2p,
    ):
        ps_sum = ps_st2p.tile([1, 512], F32, tag="ps_sum2", name="ps_sum2")
        ps_sq = ps_st2p.tile([1, 512], F32, tag="ps_sq2", name="ps_sq2")
        for k in range(KT):
            hbf = hbfp.tile([128, QBLK], BF16, tag="hbf")
            nc.vector.tensor_copy(hbf[:], h1_tiles[k][:])
            hsq = hsqp.tile([128, QBLK], BF16, tag="hsq")
            nc.vector.tensor_mul(out=hsq[:], in0=hbf[:], in1=hbf[:])
            nc.tensor.matmul(ps_sum[:], ones_bf[:], hbf[:],
                             start=(k == 0), stop=(k == KT - 1))
            nc.tensor.matmul(ps_sq[:], ones_bf[:], hsq[:],
                             start=(k == 0), stop=(k == KT - 1))
        rs_bf2, nm_bf2 = _ln_finalize(nc, stat2p, ps_sum, ps_sq, eps1, "ln2")
        rb2 = bcast2p.tile([128, QBLK], BF16)
        mb2 = bcast2p.tile([128, QBLK], BF16)
        psb = ps_st2p.tile([128, 512], F32, tag="ps_sum2", name="ps_bc2r")
        nc.tensor.matmul(psb[:], ones_row_bf[:], rs_bf2[:])
        nc.scalar.copy(out=rb2[:], in_=psb[:])
        psb2 = ps_st2p.tile([128, 512], F32, tag="ps_sq2", name="ps_bc2m")
        nc.tensor.matmul(psb2[:], ones_row_bf[:], nm_bf2[:])
        nc.scalar.copy(out=mb2[:], in_=psb2[:])
        for k in range(KT):
            t1 = ln2tp.tile([128, QBLK], BF16, tag="ln2t", name="ln2t")
            nc.vector.tensor_mul(out=t1[:], in0=h1_tiles[k][:], in1=rb2[:])
            nx2 = nx2p.tile([128, QBLK], BF16, tag="nx2")
            nc.vector.tensor_add(out=nx2[:], in0=t1[:], in1=mb2[:])
            nx2_tiles.append(nx2)


def _phase6_ffn(nc, tc, ctx, nx2_tiles):
    """FFN1 (bf16, resident W1) + SiLU; FFN2 (bf16, streamed W2) + residual."""
    W2, outT = ctx["W2"], ctx["outT"]
    w1_t, h1_tiles = ctx["w1_t"], ctx["h1_tiles"]
    b1_sb, b2_sb, hsp = ctx["b1_sb"], ctx["b2_sb"], ctx["hsp"]
    hs_tiles = []
    with tc.tile_pool(name="ps_f", bufs=3, space="PSUM") as ps_fp:
        for m in range(4 * KT):
            ms = slice(m * 128, (m + 1) * 128)
            ps = ps_fp.tile([128, QBLK], F32, tag="ps_f", name="ps_f")
            for k in range(KT):
                nc.tensor.matmul(ps[:], w1_t[k][:, ms], nx2_tiles[k][:],
                                 start=(k == 0), stop=(k == KT - 1))
            hs = hsp.tile([128, QBLK], BF16, tag="hs", name="hs")
            nc.scalar.activation(out=hs[:], in_=ps[:], func=AF.Silu,
                                 bias=b1_sb[:, m:m + 1])
            hs_tiles.append(hs)

    with (
        tc.tile_pool(name="w2", bufs=10) as w2p,
        tc.tile_pool(name="osb", bufs=2) as osbp,
        tc.tile_pool(name="ps_o", bufs=KT, space="PSUM") as ps_op,
    ):
        ps_o = [ps_op.tile([128, QBLK], F32, tag="ps_o", name=f"ps_o_{i}")
                for i in range(KT)]
        for k in range(4 * KT):
            w2 = w2p.tile([128, DIM], BF16, tag="w2", name="w2")
            nc.sync.dma_start(out=w2[:], in_=W2[k * 128:(k + 1) * 128, :])
            for m in range(KT):
                nc.tensor.matmul(ps_o[m][:], w2[:, m * 128:(m + 1) * 128],
                                 hs_tiles[k][:],
                                 start=(k == 0), stop=(k == 4 * KT - 1))
        for m in range(KT):
            osb = osbp.tile([128, QBLK], F32, tag="osb", name="osb")
            nc.vector.scalar_tensor_tensor(
                out=osb[:], in0=ps_o[m][:], scalar=b2_sb[:, m:m + 1],
                in1=h1_tiles[m][:], op0=OP.add, op1=OP.add)
            nc.sync.dma_start(out=outT[m * 128:(m + 1) * 128, :], in_=osb[:])


def _build_program():
    nc = bacc.Bacc("TRN2", target_bir_lowering=False, debug=False,
                   num_devices=NCORES)

    ctx = {}
    ctx["xT"] = nc.declare_dram_parameter("xT", [DIM, T], BF16, isOutput=False)
    ctx["xq"] = nc.declare_dram_parameter("xq", [DIM, QBLK], F32, isOutput=False)
    cosd = nc.declare_dram_parameter("cosd", [128, T], BF16, isOutput=False)
    sind = nc.declare_dram_parameter("sind", [128, T], BF16, isOutput=False)
    for nm in ("Wq8", "Wk8", "Wv8", "Wp8"):
        ctx[nm] = nc.declare_dram_parameter(nm, [NP * 128, 2 * DIM], F8,
                                            isOutput=False)
    ctx["W1"] = nc.declare_dram_parameter("W1", [DIM, 4 * DIM], BF16,
                                          isOutput=False)
    ctx["W2"] = nc.declare_dram_parameter("W2", [4 * DIM, DIM], BF16,
                                          isOutput=False)
    bq = nc.declare_dram_parameter("bq", [DIM], F32, isOutput=False)
    bk = nc.declare_dram_parameter("bk", [DIM], F32, isOutput=False)
    bp = nc.declare_dram_parameter("bp", [DIM], F32, isOutput=False)
    b1 = nc.declare_dram_parameter("b1", [4 * DIM], F32, isOutput=False)
    b2 = nc.declare_dram_parameter("b2", [DIM], F32, isOutput=False)
    ctx["outT"] = nc.declare_dram_parameter("outT", [DIM, QBLK], F32,
                                            isOutput=True)

    with TileContext(nc) as tc:
        with (
            tc.tile_pool(name="consts", bufs=1) as consts,
            tc.tile_pool(name="xqb", bufs=KT) as xqbp,
            tc.tile_pool(name="h1", bufs=KT) as h1p,
            tc.tile_pool(name="avall", bufs=1) as avallp,
            tc.tile_pool(name="wp8", bufs=NP) as wp8p,
            tc.tile_pool(name="w1", bufs=KT) as w1p,
        ):
            ones_bf = consts.tile([128, 1], BF16)
            nc.vector.memset(ones_bf[:], 1.0)
            ones_row_bf = consts.tile([1, 128], BF16)
            nc.vector.memset(ones_row_bf[:], 1.0)
            ones_row_f = consts.tile([1, HD], F32)
            nc.vector.memset(ones_row_f[:], 1.0)
            eps4 = consts.tile([NCH, 1], F32)
            nc.vector.memset(eps4[:], LN_EPS)
            eps1 = consts.tile([1, 1], F32)
            nc.vector.memset(eps1[:], LN_EPS)
            negc = consts.tile([128, 1], F32)
            nc.vector.memset(negc[:], -CSHIFT)
            bq_sb = consts.tile([128, KT], F32)
            bk_sb = consts.tile([128, KT], F32)
            bp_sb = consts.tile([128, KT], F32)
            b1_sb = consts.tile([128, 4 * KT], F32)
            b2_sb = consts.tile([128, KT], F32)
            for dram, sb in ((bq, bq_sb), (bk, bk_sb), (bp, bp_sb),
                             (b1, b1_sb), (b2, b2_sb)):
                nc.sync.dma_start(out=sb[:],
                                  in_=dram.rearrange("(a p) -> p a", p=128))
            wp8_t = []
            for t in range(NP):
                w = wp8p.tile([128, 2 * DIM], F8, tag="wp8", name=f"wp8_{t}")
                nc.sync.dma_start(out=w[:], in_=ctx["Wp8"][t * 128:(t + 1) * 128, :])
                wp8_t.append(w)

            ctx.update(ones_bf=ones_bf, ones_row_bf=ones_row_bf,
                       ones_row_f=ones_row_f, eps4=eps4, eps1=eps1, negc=negc,
                       bq_sb=bq_sb, bk_sb=bk_sb, bp_sb=bp_sb, b1_sb=b1_sb,
                       b2_sb=b2_sb, wp8_t=wp8_t, xqbp=xqbp, h1p=h1p, w1p=w1p,
                       xqb_tiles=[], h1_tiles=[], w1_t=[], k_tiles=[],
                       q_tiles=[])
            av_all = avallp.tile([128, KT * QBLK], F8)
            ctx["av_all"] = av_all

            # k/q/v span QKV..attention; cos/sin + nx span phase1..QKV
            with (
                tc.tile_pool(name="ksb", bufs=KT) as ksbp,
                tc.tile_pool(name="qsb", bufs=KT) as qsbp,
                tc.tile_pool(name="vall", bufs=1) as vallp,
            ):
                ctx["ksbp"], ctx["qsbp"] = ksbp, qsbp
                ctx["v_all"] = vallp.tile([128, RT * HEADS * (HD + 1)], F8, name="v_all")
                with (
                    tc.tile_pool(name="trig", bufs=1) as trigp,
                    tc.tile_pool(name="nxall", bufs=1) as nxallp,
                ):
                    cos_sb = trigp.tile([128, T], BF16)
                    sin_sb = trigp.tile([128, T], BF16)
                    nc.sync.dma_start(out=cos_sb[:], in_=cosd[:])
                    nc.sync.dma_start(out=sin_sb[:], in_=sind[:])
                    ctx["cos_sb"], ctx["sin_sb"] = cos_sb, sin_sb
                    ctx["nx_all"] = nxallp.tile([128, KT * T], F8, name="nx_all")
                    _phase1_ln1(nc, tc, ctx)
                    _phase2_qkv(nc, tc, ctx)
                _phase3_attention(nc, tc, ctx)
            _phase4_proj(nc, tc, ctx)
            with (
                tc.tile_pool(name="nx2", bufs=KT) as nx2p,
                tc.tile_pool(name="hs", bufs=4 * KT) as hsp,
            ):
                ctx["nx2p"], ctx["hsp"] = nx2p, hsp
                nx2_tiles = []
                _phase5_ln2(nc, tc, ctx, nx2_tiles)
                _phase6_ffn(nc, tc, ctx, nx2_tiles)

    nc.compile()
    return nc


_CACHE = {}


def _pack_dr(W, scale):
    """[1024, M] f32 -> [512, 2M] fp8e4 DoubleRow pair layout, pre-scaled."""
    M = W.shape[1]
    Wp = W.reshape(NP, 2, 128, M).transpose(0, 2, 1, 3).reshape(NP * 128, 2 * M)
    return np.clip(Wp * scale, -240.0, 240.0).astype(_f8)


def _host_prep(inputs):
    g1 = np.asarray(inputs["ln1_g"], np.float32)
    b1v = np.asarray(inputs["ln1_b"], np.float32)
    g2 = np.asarray(inputs["ln2_g"], np.float32)
    b2v = np.asarray(inputs["ln2_b"], np.float32)
    W_qkv = np.asarray(inputs["W_qkv"], np.float32)
    b_qkv = np.asarray(inputs["b_qkv"], np.float32)
    W_proj = np.asarray(inputs["W_proj"], np.float32)
    b_proj = np.asarray(inputs["b_proj"], np.float32)
    W1 = np.asarray(inputs["W_ffn1"], np.float32)
    bf1 = np.asarray(inputs["b_ffn1"], np.float32)
    W2 = np.asarray(inputs["W_ffn2"], np.float32)
    bf2 = np.asarray(inputs["b_ffn2"], np.float32)

    Wf = g1[:, None] * W_qkv
    bf = b1v @ W_qkv + b_qkv
    Wq_, Wk_, Wv_ = Wf[:, :DIM], Wf[:, DIM:2 * DIM], Wf[:, 2 * DIM:]
    bq_, bk_, bv_ = bf[:DIM], bf[DIM:2 * DIM], bf[2 * DIM:]

    perm = np.empty(HD, np.int64)
    perm[0::2] = np.arange(HD // 2)
    perm[1::2] = np.arange(HD // 2) + HD // 2
    full_perm = np.concatenate([h * HD + perm for h in range(HEADS)])
    Wq_ = Wq_[:, full_perm]; bq_ = bq_[full_perm]
    Wk_ = Wk_[:, full_perm]; bk_ = bk_[full_perm]

    inv_freq = 1.0 / (ROPE_THETA ** (np.arange(0, HD, 2, dtype=np.float32) / HD))
    pos = np.arange(T, dtype=np.float32)
    ang = pos[None, :] * inv_freq[:, None]
    cosv = np.cos(ang).astype(np.float32)
    sinv = np.sin(ang).astype(np.float32)
    cos64 = np.repeat(cosv, 2, axis=0)
    sin64 = np.repeat(sinv, 2, axis=0).copy()
    sin64[0::2] *= -1.0
    cos2 = np.concatenate([cos64, cos64], axis=0).astype(_bf)
    sin2 = np.concatenate([sin64, sin64], axis=0).astype(_bf)

    bp_eff = b_proj + bv_ @ W_proj
    W1f = g2[:, None] * W1
    b1_eff = bf1 + b2v @ W1

    c = np.ascontiguousarray
    return dict(
        Wq8=c(_pack_dr(Wq_, WS)), Wk8=c(_pack_dr(Wk_, WS)),
        Wv8=c(_pack_dr(Wv_, WS)), Wp8=c(_pack_dr(W_proj, WS)),
        W1=c(W1f.astype(_bf)), W2=c(W2.astype(_bf)),
        bq=c(bq_), bk=c(bk_), bp=c(bp_eff), b1=c(b1_eff), b2=c(bf2),
        cos2=cos2, sin2=sin2,
    )


def make_in_maps(inputs):
    P = _host_prep(inputs)
    x = np.asarray(inputs["x"], np.float32)
    shared = {k: P[k] for k in ("Wq8", "Wk8", "Wv8", "Wp8", "W1", "W2",
                                "bq", "bk", "bp", "b1", "b2")}
    in_maps = []
    for c in range(NCORES):
        b = c // CPB
        qb = c % CPB
        roll = -qb * QBLK
        xTr = np.ascontiguousarray(np.roll(x[b].T, roll, axis=1))
        cosd = np.ascontiguousarray(np.roll(P["cos2"], roll, axis=1))
        sind = np.ascontiguousarray(np.roll(P["sin2"], roll, axis=1))
        in_maps.append(dict(shared, xT=xTr.astype(_bf),
                            xq=np.ascontiguousarray(xTr[:, 0:QBLK]),
                            cosd=cosd, sind=sind))
    return in_maps


def assemble_out(results):
    out = np.empty((B, T, DIM), np.float32)
    for c in range(NCORES):
        b = c // CPB
        qb = c % CPB
        out[b, qb * QBLK:(qb + 1) * QBLK, :] = results[c]["outT"].T
    return out


def get_program():
    if "nc" not in _CACHE:
        _CACHE["nc"] = _build_program()
    return _CACHE["nc"]


def kernel(**inputs):
    nc = get_program()
    in_maps = make_in_maps(inputs)
    res = run_bass_kernel_spmd(nc, in_maps, list(range(NCORES)))
    return assemble_out(res.results)
